# revision 1
# baseline (speedup 1.0000x reference)
"""CRF negative-log-likelihood loss kernel for Trainium2 (8 NeuronCores).

Problem: B=256, S=2048, T=64 CRF loss (torchcrf-style), mask all-ones.

Strategy
--------
Data-parallel over batch: each of the 8 cores gets 32 batch rows.

Denominator (log-partition): forward/backward meet-in-the-middle.  The
forward chain  E_p = X_p * (W^T E_{p-1})  and the backward chain
C_s = X_s * (W C_{s+1})  (exp domain, X_s = exp(em_s - c0), W =
exp(trans)) advance together: one 128x128 block-diagonal matmul (top
block W, bottom block W^T as lhsT, bf16) + one [128,32] DVE multiply
per round.  1023 rounds instead of 2047.  Z = E_{S/2-1} . (W C_{S/2}).
A constant per-step prescale c0 plus exact renormalization every RN
rounds keeps fp32 in range; the actually-applied bf16 reciprocals are
logged exactly (c_acc -= ln(rhat)) and added back at the end:
  den = ln(Zt) + c_f + c_b + S*c0.

Numerator (summed over the core's batch): one-hot matmuls, all bf16
(one-hots are exact in bf16; em is bf16-rounded, error ~1e-6 relative).
  M = sum_{b,s} onehot_{b,s} x em_{b,s}      -> trace(M) = sum em[b,s,tag]
  N = sum_{b,s} onehot_{b,s} x onehot_{b,s+1} -> <N, trans> = sum trans[tag,tagnext]
Shifted one-hots come from a host-shifted tag array (pad -1 -> zero
row).  start/end transitions are folded into em rows s=0 / s=S-1 on the
host, which also makes X_0 / X_{S-1} the correct chain initializers.

Emissions travel as bf16 (half the DMA bytes); exp() output X stays
f32.  X chunks are stored j-major ([128, j, b]) so the per-round DVE
read is contiguous.

Per-core outputs: den[1,32] f32, misc[1,2] f32 = (em part incl
start/end, trans part).  Host: loss = -(sum(misc) - sum(den)) / B.
"""

import contextlib

import numpy as np
import ml_dtypes

F32_NP = np.float32
BF16_NP = ml_dtypes.bfloat16

B, S, T = 256, 2048, 64
NCORES = 8
BSH = B // NCORES  # 32
CHUNK = 128
C0 = 4.8204  # ~ ln(64 * e^0.5 * sinh(1)) : expected per-step log growth
RN = 24  # renorm every RN rounds

_NC_CACHE = {}


def build(n_chunks=16, bsh=BSH, nrep=1, fake_x=False, no_num=False,
          no_rounds=False, rn=RN, pround_bufs=4, spool_bufs=6,
          dma_all=False, fake_x_dma=False, no_mn=False, mn_banks=1):
    """Build + compile the per-core Bass module. n_chunks*128 = seq len.

    nrep>1 wraps the whole computation in a device-side loop (timing
    only); fake_x / no_num / no_rounds strip parts for cost bisection."""
    import concourse.bacc as bacc
    import concourse.mybir as mybir
    import concourse.tile as tile

    F32 = mybir.dt.float32
    BF16 = mybir.dt.bfloat16
    AF = mybir.ActivationFunctionType
    ALU = mybir.AluOpType

    s_len = n_chunks * CHUNK
    half = n_chunks // 2
    assert half * 2 == n_chunks and half >= 1
    n_rounds = half * CHUNK - 1

    nc = bacc.Bacc("TRN2", target_bir_lowering=False, debug=False,
                   num_devices=NCORES)

    em_x_d = nc.dram_tensor("emx", [half, 128, 128, bsh], BF16,
                            kind="ExternalInput")
    em_m_d = nc.dram_tensor("emm", [n_chunks, 128, bsh, T], BF16,
                            kind="ExternalInput")
    tags_d = nc.dram_tensor("tagsf", [bsh, s_len], F32, kind="ExternalInput")
    tagsq_d = nc.dram_tensor("tagsq", [bsh, s_len], F32, kind="ExternalInput")
    trans_d = nc.dram_tensor("trans", [T, T], F32, kind="ExternalInput")
    bones_d = nc.dram_tensor("bones", [128, 2], BF16, kind="ExternalInput")
    bsel_d = nc.dram_tensor("bsel", [2, 128], BF16, kind="ExternalInput")
    iota_d = nc.dram_tensor("iotat", [128, T], BF16, kind="ExternalInput")
    ident_d = nc.dram_tensor("ident", [128, 128], F32, kind="ExternalInput")
    den_d = nc.dram_tensor("den", [1, bsh], F32, kind="ExternalOutput")
    misc_d = nc.dram_tensor("misc", [1, 2], F32, kind="ExternalOutput")

    with tile.TileContext(nc) as tc, nc.allow_low_precision(
            reason="bf16 state/weights validated against f64 reference"):
        with (
            tc.tile_pool(name="consts", bufs=1) as consts,
            tc.tile_pool(name="xchunk", bufs=3) as xpool,
            tc.tile_pool(name="xraw", bufs=3) as xrawpool,
            tc.tile_pool(name="emt", bufs=8) as empool,
            tc.tile_pool(name="ot", bufs=4 * bsh + 16) as opool,
            tc.tile_pool(name="state", bufs=spool_bufs) as spool,
            tc.tile_pool(name="small", bufs=4) as smallpool,
            tc.tile_pool(name="pround", bufs=pround_bufs,
                         space="PSUM") as pround,
            tc.tile_pool(name="pacc", bufs=1, space="PSUM") as pacc,
            tc.tile_pool(name="pmisc", bufs=1, space="PSUM") as pmisc,
        ):
            rep_ctx = (tc.For_i(0, nrep, 1) if nrep > 1
                       else contextlib.nullcontext())
            with rep_ctx:
                # ---------------- constants / setup ----------------
                ident = consts.tile([128, 128], F32, tag="ident")
                nc.sync.dma_start(ident[:], ident_d.ap())
                iota_t = consts.tile([128, T], BF16, tag="iota")
                nc.sync.dma_start(iota_t[:], iota_d.ap())
                trans_sb = consts.tile([T, T], F32, tag="trans")
                nc.sync.dma_start(trans_sb[:], trans_d.ap())

                # block-diagonal lhsT (bf16): top-left W (for W^T @ E),
                # bottom-right W^T (for W @ C)
                blockw = consts.tile([128, 128], BF16, tag="blockw")
                nc.vector.memset(blockw[:], 0.0)
                nc.scalar.activation(blockw[0:T, 0:T], trans_sb[:], AF.Exp)
                tp = pmisc.tile([128, 128], F32, tag="m128")
                nc.tensor.matmul(tp[0:T, 0:T], trans_sb[:], ident[0:T, 0:T],
                                 start=True, stop=True)
                nc.scalar.activation(blockw[T:128, T:128], tp[0:T, 0:T],
                                     AF.Exp)

                blockones = consts.tile([128, 2], BF16, tag="blockones")
                nc.sync.dma_start(blockones[:], bones_d.ap())
                blocksel = consts.tile([2, 128], BF16, tag="blocksel")
                nc.sync.dma_start(blocksel[:], bsel_d.ap())
                ones64 = consts.tile([T, 1], F32, tag="ones64")
                nc.vector.memset(ones64[:], 1.0)
                ones2 = consts.tile([2, 1], F32, tag="ones2")
                nc.vector.memset(ones2[:], 1.0)
                negc0 = consts.tile([128, 1], F32, tag="negc0")
                nc.vector.memset(negc0[:], -C0)

                c_acc = consts.tile([2, bsh], F32, tag="cacc")
                nc.vector.memset(c_acc[:], 0.0)

                # tag tiles: tile[p, g] = tags[b, 128g + p]
                tag_tiles, tagq_tiles = [], []
                for b in range(bsh):
                    tt = consts.tile([128, n_chunks], F32, tag=f"tags{b}")
                    nc.sync.dma_start(
                        tt[:],
                        tags_d.ap()[b].rearrange("(g p) -> p g", p=128))
                    tag_tiles.append(tt)
                    tq = consts.tile([128, n_chunks], F32, tag=f"tagsq{b}")
                    nc.sync.dma_start(
                        tq[:],
                        tagsq_d.ap()[b].rearrange("(g p) -> p g", p=128))
                    tagq_tiles.append(tq)

                # numerator PSUM accumulators (alive whole kernel)
                m_ps = pacc.tile([T, T], F32, tag="m_ps")
                n_ps = pacc.tile([T, T], F32, tag="n_ps")
                if mn_banks == 2:
                    m_ps2 = pacc.tile([T, T], F32, tag="m_ps2")
                    n_ps2 = pacc.tile([T, T], F32, tag="n_ps2")

                num_state = {"m_first": True, "n_first": True,
                             "m_last": None, "n_last": None}

                emg = {}      # em-chunk g -> tile [128, bsh, T] bf16
                exraw = {}    # x-chunk c -> tile [128, 128, bsh] bf16
                ohots = {}    # em-chunk g -> (O list, Oq list)

                def dma_chunk(d):
                    xr = xrawpool.tile([128, 128, bsh], BF16, tag="xr")
                    nc.sync.dma_start(xr[:], em_x_d.ap()[d])
                    exraw[d] = xr
                    for g in (d, n_chunks - 1 - d):
                        eg = empool.tile([128, bsh, T], BF16, tag="em")
                        nc.sync.dma_start(eg[:], em_m_d.ap()[g])
                        emg[g] = eg

                def build_onehots(d):
                    if no_num:
                        return
                    for g in (d, n_chunks - 1 - d):
                        os_, oqs = [], []
                        for b in range(bsh):
                            ot = opool.tile([128, T], BF16, tag="o")
                            nc.gpsimd.tensor_scalar(
                                ot[:], iota_t[:],
                                tag_tiles[b][:, g:g + 1], None,
                                op0=ALU.is_equal)
                            oq = opool.tile([128, T], BF16, tag="oq")
                            nc.gpsimd.tensor_scalar(
                                oq[:], iota_t[:],
                                tagq_tiles[b][:, g:g + 1], None,
                                op0=ALU.is_equal)
                            os_.append(ot)
                            oqs.append(oq)
                        ohots[g] = (os_, oqs)

                def mn_quanta(d):
                    """Per-(g,b) numerator matmul quanta for chunk d."""
                    qs = []
                    if no_num:
                        return qs
                    for g in (d, n_chunks - 1 - d):
                        def mk(g):
                            def done(_b):
                                del emg[g]
                            return done
                        for b in range(bsh):
                            def q(g=g, b=b, fin=(mk(g) if b == bsh - 1
                                                 else None)):
                                os_, oqs = ohots[g]
                                if not no_mn:
                                    mt = (m_ps if (mn_banks == 1 or b % 2
                                                   == 0) else m_ps2)
                                    nt = (n_ps if (mn_banks == 1 or b % 2
                                                   == 0) else n_ps2)
                                    key = ("m_first" if mt is m_ps
                                           else "m2_first")
                                    mm = nc.tensor.matmul(
                                        mt[:], os_[b][:], emg[g][:, b, :],
                                        start=num_state.get(key, True),
                                        stop=False, skip_group_check=True)
                                    num_state[key] = False
                                    num_state["m_last" if mt is m_ps
                                              else "m2_last"] = mm
                                    keyn = ("n_first" if nt is n_ps
                                            else "n2_first")
                                    nn_ = nc.tensor.matmul(
                                        nt[:], os_[b][:], oqs[b][:],
                                        start=num_state.get(keyn, True),
                                        stop=False, skip_group_check=True)
                                    num_state[keyn] = False
                                    num_state["n_last" if nt is n_ps
                                              else "n2_last"] = nn_
                                if fin is not None:
                                    fin(b)
                                    del ohots[g]
                            qs.append(q)
                    return qs

                def x_quanta(c):
                    """ACT-exp quanta producing X chunk c from em_x."""
                    xc = xpool.tile([128, 128, bsh], F32, tag="xc")
                    if fake_x or fake_x_dma:
                        def q():
                            nc.gpsimd.memset(xc[:], 0.0133)
                        return xc, [q]
                    qs = []
                    for hj in range(4):
                        def q(hj=hj):
                            sl = slice(hj * 32, (hj + 1) * 32)
                            nc.scalar.activation(
                                xc[:, sl, :], exraw[c][:, sl, :],
                                AF.Exp, bias=negc0[:])
                            if hj == 3:
                                del exraw[c]
                        qs.append(q)
                    return xc, qs

                # ---------------- main pipeline ----------------
                # priming: chunks 0 and 1 fully ready; em for 2 in flight
                from collections import deque
                bg = deque()
                xchunks = {}
                if fake_x:
                    xc, qs = x_quanta(0)
                    [q() for q in qs]
                    xchunks[0] = xc
                    if half > 1:
                        xc, qs = x_quanta(1)
                        [q() for q in qs]
                        xchunks[1] = xc
                else:
                    dma_chunk(0)
                    if half > 1:
                        dma_chunk(1)
                    if dma_all:
                        for d in range(2, half):
                            dma_chunk(d)
                    elif half > 2:
                        dma_chunk(2)
                    build_onehots(0)
                    xc, qs = x_quanta(0)
                    [q() for q in qs]
                    xchunks[0] = xc
                    if half > 1:
                        xc, qs = x_quanta(1)
                        [q() for q in qs]
                        xchunks[1] = xc

                state = spool.tile([128, bsh], BF16, tag="st")
                nc.gpsimd.tensor_copy(state[:], xchunks[0][:, 0, :])

                r_end = 0 if no_rounds else n_rounds
                for r in range(1, r_end + 1):
                    c, j = r >> 7, r & 127
                    if j == 1:
                        if not fake_x and not dma_all and c + 3 <= half - 1:
                            dma_chunk(c + 3)
                        if not fake_x and c + 1 <= half - 1:
                            build_onehots(c + 1)
                        if c + 2 <= half - 1:
                            xc, qs = x_quanta(c + 2)
                            xchunks[c + 2] = xc
                            bg.extend(qs)
                            xchunks.pop(c - 1, None)
                        if not fake_x:
                            bg.extend(mn_quanta(c))
                    if bg:
                        bg.popleft()()
                    p = pround.tile([128, bsh], F32, tag="p")
                    nc.tensor.matmul(p[:], blockw[:], state[:],
                                     start=True, stop=True)
                    state = spool.tile([128, bsh], BF16, tag="st")
                    nc.vector.tensor_mul(state[:], p[:], xchunks[c][:, j, :])

                    if r % rn == 0 and r < n_rounds:
                        mass = pmisc.tile([2, bsh], F32, tag="m2")
                        nc.tensor.matmul(mass[:], blockones[:], state[:],
                                         start=True, stop=True)
                        rmass = smallpool.tile([2, bsh], BF16, tag="rm")
                        nc.vector.reciprocal(rmass[:], mass[:])
                        lnr = smallpool.tile([2, bsh], F32, tag="lnr")
                        nc.scalar.activation(lnr[:], rmass[:], AF.Ln)
                        nc.gpsimd.tensor_sub(c_acc[:], c_acc[:], lnr[:])
                        rbc = pmisc.tile([128, 128], F32, tag="m128")
                        nc.tensor.matmul(rbc[:, 0:bsh], blocksel[:],
                                         rmass[:], start=True, stop=True)
                        nstate = spool.tile([128, bsh], BF16, tag="st")
                        nc.vector.tensor_mul(nstate[:], state[:],
                                             rbc[:, 0:bsh])
                        state = nstate

                while bg:
                    bg.popleft()()
                if no_rounds and not fake_x:
                    for q in mn_quanta(0):
                        q()

                # ---------------- final combine ----------------
                # beta = W @ C on partitions 0..63 (aligned base-64 matmul)
                pf = pround.tile([128, bsh], F32, tag="p")
                nc.tensor.matmul(pf[0:T, :], blockw[T:128, T:128],
                                 state[T:128, :], start=True, stop=True)
                y = smallpool.tile([T, bsh], F32, tag="y")
                nc.vector.tensor_mul(y[:], state[0:T, :], pf[0:T, :])
                z = pmisc.tile([2, bsh], F32, tag="m2")
                nc.tensor.matmul(z[0:1, :], ones64[:], y[:],
                                 start=True, stop=True)
                den_sb = smallpool.tile([1, bsh], F32, tag="densb")
                nc.scalar.activation(den_sb[:], z[0:1, :], AF.Ln)
                csum = pmisc.tile([2, bsh], F32, tag="m2")
                nc.tensor.matmul(csum[0:1, :], ones2[:], c_acc[:],
                                 start=True, stop=True)
                csum_sb = smallpool.tile([1, bsh], F32, tag="csum")
                nc.scalar.activation(csum_sb[:], csum[0:1, :], AF.Copy)
                nc.gpsimd.tensor_add(den_sb[:], den_sb[:], csum_sb[:])
                nc.gpsimd.tensor_scalar_add(den_sb[:], den_sb[:],
                                            float(s_len) * C0)
                nc.sync.dma_start(den_d.ap(), den_sb[:])

                # numerator finish
                if num_state["m_last"] is None:
                    misc_sbz = smallpool.tile([1, 2], F32, tag="miscsb")
                    nc.vector.memset(misc_sbz[:], 0.0)
                    nc.sync.dma_start(misc_d.ap(), misc_sbz[:])
                else:
                    num_state["m_last"].ins.stop_tensor_calc = True
                    num_state["n_last"].ins.stop_tensor_calc = True
                    if mn_banks == 2:
                        num_state["m2_last"].ins.stop_tensor_calc = True
                        num_state["n2_last"].ins.stop_tensor_calc = True
                        mps2sb = smallpool.tile([T, T], F32, tag="scr")
                        nc.vector.tensor_copy(mps2sb[:], m_ps2[:])
                        nps2sb = smallpool.tile([T, T], F32, tag="scr2")
                        nc.vector.tensor_copy(nps2sb[:], n_ps2[:])
                    scr = smallpool.tile([T, T], F32, tag="scr")
                    acc2 = smallpool.tile([T, 2], F32, tag="acc2")
                    nc.vector.scalar_tensor_tensor(
                        scr[:], ident[0:T, 0:T], 1.0, m_ps[:],
                        op0=ALU.bypass, op1=ALU.mult, accum_out=acc2[:, 0:1])
                    scr2 = smallpool.tile([T, T], F32, tag="scr2")
                    nc.vector.scalar_tensor_tensor(
                        scr2[:], trans_sb[:], 1.0, n_ps[:],
                        op0=ALU.bypass, op1=ALU.mult, accum_out=acc2[:, 1:2])
                    if mn_banks == 2:
                        acc2b = smallpool.tile([T, 2], F32, tag="acc2b")
                        nc.vector.scalar_tensor_tensor(
                            mps2sb[:], ident[0:T, 0:T], 1.0, m_ps2[:],
                            op0=ALU.bypass, op1=ALU.mult,
                            accum_out=acc2b[:, 0:1])
                        nc.vector.scalar_tensor_tensor(
                            nps2sb[:], trans_sb[:], 1.0, n_ps2[:],
                            op0=ALU.bypass, op1=ALU.mult,
                            accum_out=acc2b[:, 1:2])
                        nc.vector.tensor_add(acc2[:], acc2[:], acc2b[:])
                    misc_ps = pmisc.tile([2, bsh], F32, tag="m2")
                    nc.tensor.matmul(misc_ps[0:1, 0:2], ones64[:], acc2[:],
                                     start=True, stop=True)
                    misc_sb = smallpool.tile([1, 2], F32, tag="miscsb")
                    nc.scalar.activation(misc_sb[:], misc_ps[0:1, 0:2],
                                         AF.Copy)
                    nc.sync.dma_start(misc_d.ap(), misc_sb[:])

    nc.compile()
    return nc


def _get_nc(n_chunks=16, bsh=BSH):
    key = (n_chunks, bsh)
    if key not in _NC_CACHE:
        _NC_CACHE[key] = build(n_chunks, bsh)
    return _NC_CACHE[key]


def _consts():
    iota = np.broadcast_to(np.arange(T, dtype=F32_NP),
                           (128, T)).astype(BF16_NP)
    ident = np.eye(128, dtype=F32_NP)
    bones = np.zeros((128, 2), dtype=F32_NP)
    bones[0:T, 0] = 1.0
    bones[T:128, 1] = 1.0
    bsel = np.zeros((2, 128), dtype=F32_NP)
    bsel[0, 0:T] = 1.0
    bsel[1, T:128] = 1.0
    return iota, ident, bones.astype(BF16_NP), bsel.astype(BF16_NP)


def _shift_tags(tags_f):
    tq = np.empty_like(tags_f)
    tq[:, :-1] = tags_f[:, 1:]
    tq[:, -1] = -1.0
    return tq


def make_in_maps(emissions, start_transitions, end_transitions, transitions,
                 tags, ncores=NCORES):
    """Host prep: fold start/end into em, convert to bf16, build the two
    DMA-friendly layouts (em_x for the recurrence, em_m for the
    numerator), shard over cores."""
    em = np.asarray(emissions, dtype=F32_NP).copy()
    em[:, 0, :] += np.asarray(start_transitions, dtype=F32_NP)
    em[:, -1, :] += np.asarray(end_transitions, dtype=F32_NP)
    em_b = em.astype(BF16_NP)
    b_all, s_len = em.shape[0], em.shape[1]
    n_chunks = s_len // CHUNK
    half = n_chunks // 2
    # em_x[c, row, j, b]: rows 0:64 fwd t of chunk c (s = 128c + j);
    # rows 64:128 bwd t of chunk n_chunks-1-c with j reversed
    # (s = s_len-1 - 128c - j)
    fwd = em_b[:, :half * 128, :].reshape(b_all, half, 128, T)
    fwd = fwd.transpose(1, 3, 2, 0)                    # [c, t, j, b]
    bwd = em_b[:, half * 128:, :].reshape(b_all, half, 128, T)
    bwd = bwd[:, ::-1, ::-1, :].transpose(1, 3, 2, 0)  # [c, t, j, b]
    em_x = np.concatenate([fwd, bwd], axis=1)          # [c, 128, 128, b]
    # em_m[g, s, b, t] (natural order per chunk)
    em_m = em_b.reshape(b_all, n_chunks, 128, T).transpose(1, 2, 0, 3)
    tags_f = np.asarray(tags).astype(F32_NP).reshape(b_all, s_len)
    tags_b = np.ascontiguousarray(tags_f)
    tagsq_b = np.ascontiguousarray(_shift_tags(tags_f))
    trans = np.asarray(transitions, dtype=F32_NP).reshape(T, T)
    iota, ident, bones, bsel = _consts()
    bsh = b_all // ncores
    in_maps = []
    for cidx in range(ncores):
        sl = slice(cidx * bsh, (cidx + 1) * bsh)
        in_maps.append({
            "emx": np.ascontiguousarray(em_x[:, :, :, sl]),
            "emm": np.ascontiguousarray(em_m[:, :, sl, :]),
            "tagsf": tags_b[sl],
            "tagsq": tagsq_b[sl],
            "trans": trans,
            "bones": bones,
            "bsel": bsel,
            "iotat": iota,
            "ident": ident,
        })
    return in_maps


def kernel(emissions, start_transitions, end_transitions, transitions,
           tags, mask):
    """Full-input entry point; shards over 8 NeuronCores internally."""
    from concourse.bass_utils import run_bass_kernel_spmd

    emissions = np.asarray(emissions)
    assert emissions.shape == (B, S, T)
    assert (np.asarray(mask) != 0).all(), "kernel assumes all-ones mask"

    in_maps = make_in_maps(emissions, start_transitions, end_transitions,
                           transitions, tags)
    nc = _get_nc()
    res = run_bass_kernel_spmd(nc, in_maps, core_ids=list(range(NCORES)))

    num_total = 0.0
    den_total = 0.0
    for cidx in range(NCORES):
        r = res.results[cidx]
        num_total += float(r["misc"].sum())
        den_total += float(r["den"].sum())
    loss = -(num_total - den_total) / float(B)
    return np.float32(loss)



# revision 4
# speedup vs baseline: 3.9511x; 3.9511x over previous
"""CRF negative-log-likelihood loss kernel for Trainium2 (8 NeuronCores).

Problem: B=256, S=2048, T=64 CRF loss (torchcrf-style), mask all-ones.

Strategy (v2)
-------------
Data-parallel over batch: each of the 8 cores gets 32 batch rows.

Denominator (log-partition): forward/backward meet-in-the-middle in the
exp domain.  One 128x128 block-diagonal matmul (top block W, bottom
block W^T as lhsT, bf16) + one [128,32] DVE multiply per round; 1023
rounds.  The serial MM->DVE->MM latency chain is the wall-clock floor,
so everything else is kept OFF the two hot engines' queues:

 * numerator tag-gather: onehots are built on the HOST and DMA'd in;
   one Pool-engine scalar_tensor_tensor per 128-step chunk reduces
   (em (*) onehot) into a per-chunk accumulator column.  No PE or DVE
   involvement.
 * numerator transition part: sum_{s} trans[tag_s, tag_{s+1}] depends
   only on tags -> host bincount, exact in f64.
 * renorm every RN rounds: bf16 reciprocal applied to the state; the
   exact applied factors are accumulated as an f32 PRODUCT on DVE (no
   Ln on ACT -> no activation-table thrash), logged once at the end:
       den = ln(Zt) - ln(Pf) - ln(Pb) + S*c0.

Emissions travel as bf16 (em_x for the chain, em_m + onehots for the
numerator).  exp() runs on ACT in 4 quanta per chunk, 2 chunks ahead.

Per-core outputs: den[1,32] f32, numacc[128,16] f32 (per-chunk gather
partial sums; host reduces).  Host: loss = -(em_part + trans_part
- sum(den)) / B.
"""

import contextlib

import numpy as np
import ml_dtypes

F32_NP = np.float32
BF16_NP = ml_dtypes.bfloat16

B, S, T = 256, 2048, 64
NCORES = 8
BSH = B // NCORES  # 32
CHUNK = 128
C0 = 4.8204  # ~ ln(64 * e^0.5 * sinh(1)) : expected per-step log growth
RN = 64  # renorm every RN rounds

_NC_CACHE = {}


def build(n_chunks=16, bsh=BSH, nrep=1, fake_x=False, no_num=False,
          no_rounds=False, rn=RN, pround_bufs=4, spool_bufs=6,
          split64=False, warm=0):
    """Build + compile the per-core Bass module. n_chunks*128 = seq len.

    nrep>1 wraps the whole computation in a device-side loop (timing
    only); fake_x / no_num / no_rounds strip parts for cost bisection.
    split64: use two 64-contraction quadrant matmuls per round instead
    of one 128x128 block-diagonal matmul (shorter PE drain).
    warm: issue `warm` dummy matmuls per round to keep the PE HAM
    un-throttled (2.4 GHz)."""
    import concourse.bacc as bacc
    import concourse.mybir as mybir
    import concourse.tile as tile

    F32 = mybir.dt.float32
    BF16 = mybir.dt.bfloat16
    AF = mybir.ActivationFunctionType
    ALU = mybir.AluOpType

    s_len = n_chunks * CHUNK
    half = n_chunks // 2
    assert half * 2 == n_chunks and half >= 1
    n_rounds = half * CHUNK - 1

    nc = bacc.Bacc("TRN2", target_bir_lowering=False, debug=False,
                   num_devices=NCORES)

    em_x_d = nc.dram_tensor("emx", [half, 128, 128, bsh], BF16,
                            kind="ExternalInput")
    em_m_d = nc.dram_tensor("emm", [n_chunks, 128, bsh, T], BF16,
                            kind="ExternalInput")
    oh_d = nc.dram_tensor("oh", [n_chunks, 128, bsh, T], BF16,
                          kind="ExternalInput")
    blockw_d = nc.dram_tensor("blockw", [128, 128], BF16,
                              kind="ExternalInput")
    bones_d = nc.dram_tensor("bones", [128, 2], BF16, kind="ExternalInput")
    bsel_d = nc.dram_tensor("bsel", [2, 128], BF16, kind="ExternalInput")
    den_d = nc.dram_tensor("den", [1, bsh], F32, kind="ExternalOutput")
    num_d = nc.dram_tensor("numacc", [128, n_chunks], F32,
                           kind="ExternalOutput")

    with tile.TileContext(nc) as tc, nc.allow_low_precision(
            reason="bf16 state/weights validated against f64 reference"):
        with (
            tc.tile_pool(name="consts", bufs=1) as consts,
            tc.tile_pool(name="xchunk", bufs=3) as xpool,
            tc.tile_pool(name="xraw", bufs=3) as xrawpool,
            tc.tile_pool(name="emt", bufs=4) as empool,
            tc.tile_pool(name="oht", bufs=4) as ohpool,
            tc.tile_pool(name="scr", bufs=2) as scrpool,
            tc.tile_pool(name="state", bufs=spool_bufs) as spool,
            tc.tile_pool(name="rm", bufs=3) as rmpool,
            tc.tile_pool(name="small", bufs=4) as smallpool,
            tc.tile_pool(name="pround", bufs=pround_bufs,
                         space="PSUM") as pround,
            tc.tile_pool(name="pmisc", bufs=2, space="PSUM") as pmisc,
            tc.tile_pool(name="pwarm", bufs=1, space="PSUM") as pwarm,
        ):
            rep_ctx = (tc.For_i(0, nrep, 1) if nrep > 1
                       else contextlib.nullcontext())
            with rep_ctx:
                # ---------------- constants / setup ----------------
                blockw = consts.tile([128, 128], BF16, tag="blockw")
                nc.sync.dma_start(blockw[:], blockw_d.ap())
                blockones = consts.tile([128, 2], BF16, tag="blockones")
                nc.sync.dma_start(blockones[:], bones_d.ap())
                blocksel = consts.tile([2, 128], BF16, tag="blocksel")
                nc.sync.dma_start(blocksel[:], bsel_d.ap())
                ones64 = consts.tile([T, 1], F32, tag="ones64")
                nc.vector.memset(ones64[:], 1.0)
                ones2 = consts.tile([2, 1], F32, tag="ones2")
                nc.vector.memset(ones2[:], 1.0)
                negc0 = consts.tile([128, 1], F32, tag="negc0")
                nc.vector.memset(negc0[:], -C0)

                pacc = consts.tile([2, bsh], F32, tag="pacc")
                nc.vector.memset(pacc[:], 1.0)
                numacc = consts.tile([128, n_chunks], F32, tag="numacc")
                nc.vector.memset(numacc[:], 0.0)

                emg = {}      # em-chunk g -> tile [128, bsh, T] bf16
                ohg = {}      # onehot-chunk g -> tile [128, bsh, T] bf16
                exraw = {}    # x-chunk c -> tile [128, 128, bsh] bf16

                def dma_chunk(d):
                    xr = xrawpool.tile([128, 128, bsh], BF16, tag="xr")
                    nc.sync.dma_start(xr[:], em_x_d.ap()[d])
                    exraw[d] = xr
                    if no_num:
                        return
                    for g in (d, n_chunks - 1 - d):
                        eg = empool.tile([128, bsh, T], BF16, tag="em")
                        nc.sync.dma_start(eg[:], em_m_d.ap()[g])
                        emg[g] = eg
                        og = ohpool.tile([128, bsh, T], BF16, tag="oh")
                        nc.sync.dma_start(og[:], oh_d.ap()[g])
                        ohg[g] = og

                def num_chunk(d):
                    """Pool-engine gather-reduce for both chunks of pair d."""
                    if no_num:
                        return
                    for g in (d, n_chunks - 1 - d):
                        scr = scrpool.tile([128, bsh, T], BF16, tag="scr")
                        nc.gpsimd.tensor_tensor(
                            scr[:], emg[g][:], ohg[g][:], op=ALU.mult)
                        nc.gpsimd.tensor_reduce(
                            numacc[0:1, g:g + 1], scr[:],
                            axis=mybir.AxisListType.XYZWC, op=ALU.add)
                        del emg[g], ohg[g]

                def x_quanta(c):
                    """ACT-exp quanta producing X chunk c from em_x."""
                    xc = xpool.tile([128, 128, bsh], F32, tag="xc")
                    if fake_x:
                        def q():
                            nc.gpsimd.memset(xc[:], 0.0133)
                        return xc, [q]
                    qs = []
                    for hj in range(4):
                        def q(hj=hj):
                            sl = slice(hj * 32, (hj + 1) * 32)
                            nc.scalar.activation(
                                xc[:, sl, :], exraw[c][:, sl, :],
                                AF.Exp, bias=negc0[:])
                            if hj == 3:
                                del exraw[c]
                        qs.append(q)
                    return xc, qs

                # ---------------- main pipeline ----------------
                from collections import deque
                bg = deque()
                xchunks = {}
                if fake_x:
                    for c in (0, 1):
                        if c <= half - 1:
                            xc, qs = x_quanta(c)
                            [q() for q in qs]
                            xchunks[c] = xc
                else:
                    dma_chunk(0)
                    if half > 1:
                        dma_chunk(1)
                    if half > 2:
                        dma_chunk(2)
                    num_chunk(0)
                    xc, qs = x_quanta(0)
                    [q() for q in qs]
                    xchunks[0] = xc
                    if half > 1:
                        xc, qs = x_quanta(1)
                        [q() for q in qs]
                        xchunks[1] = xc

                state = spool.tile([128, bsh], BF16, tag="st")
                nc.vector.tensor_copy(state[:], xchunks[0][:, 0, :])

                if warm:
                    wsrc = consts.tile([128, 8], BF16, tag="wsrc")
                    nc.vector.memset(wsrc[:], 0.5)
                    wps = pwarm.tile([128, 8], F32, tag="wps")

                r_end = 0 if no_rounds else n_rounds
                for r in range(1, r_end + 1):
                    c, j = r >> 7, r & 127
                    if j == 1:
                        if not fake_x and c + 3 <= half - 1:
                            dma_chunk(c + 3)
                        if c + 2 <= half - 1:
                            xc, qs = x_quanta(c + 2)
                            xchunks[c + 2] = xc
                            bg.extend(qs)
                            xchunks.pop(c - 1, None)
                        if not fake_x and c + 1 <= half - 1:
                            num_chunk(c + 1)
                    if bg:
                        bg.popleft()()
                    for _ in range(warm):
                        nc.tensor.matmul(wps[:], blockw[:, 0:8], wsrc[:],
                                         start=True, stop=True,
                                         skip_group_check=True)
                    p = pround.tile([128, bsh], F32, tag="p")
                    if split64:
                        nc.tensor.matmul(p[0:T, :], blockw[0:T, 0:T],
                                         state[0:T, :], start=True,
                                         stop=True)
                        nc.tensor.matmul(p[T:128, :], blockw[T:128, T:128],
                                         state[T:128, :], start=True,
                                         stop=True)
                    else:
                        nc.tensor.matmul(p[:], blockw[:], state[:],
                                         start=True, stop=True)
                    state = spool.tile([128, bsh], BF16, tag="st")
                    nc.vector.tensor_mul(state[:], p[:], xchunks[c][:, j, :])

                    if r % rn == 0 and r < n_rounds:
                        mass = pmisc.tile([2, bsh], F32, tag="m2")
                        nc.tensor.matmul(mass[:], blockones[:], state[:],
                                         start=True, stop=True)
                        rmass = rmpool.tile([2, bsh], BF16, tag="rm")
                        nc.vector.reciprocal(rmass[:], mass[:])
                        nc.vector.tensor_mul(pacc[:], pacc[:], rmass[:])
                        rbc = pmisc.tile([128, bsh], F32, tag="rbc")
                        nc.tensor.matmul(rbc[:], blocksel[:], rmass[:],
                                         start=True, stop=True)
                        nstate = spool.tile([128, bsh], BF16, tag="st")
                        nc.vector.tensor_mul(nstate[:], state[:], rbc[:])
                        state = nstate

                while bg:
                    bg.popleft()()
                if no_rounds and not fake_x:
                    for d in range(1, half):
                        num_chunk(d)

                # ---------------- final combine ----------------
                # beta = W @ C on partitions 0..63 (aligned base-64 matmul)
                pf = pround.tile([128, bsh], F32, tag="p")
                nc.tensor.matmul(pf[0:T, :], blockw[T:128, T:128],
                                 state[T:128, :], start=True, stop=True)
                y = smallpool.tile([T, bsh], F32, tag="y")
                nc.vector.tensor_mul(y[:], state[0:T, :], pf[0:T, :])
                z = pmisc.tile([2, bsh], F32, tag="m2")
                nc.tensor.matmul(z[0:1, :], ones64[:], y[:],
                                 start=True, stop=True)
                den_sb = smallpool.tile([1, bsh], F32, tag="densb")
                nc.scalar.activation(den_sb[:], z[0:1, :], AF.Ln)
                lp = smallpool.tile([2, bsh], F32, tag="lp")
                nc.scalar.activation(lp[:], pacc[:], AF.Ln)
                csum = pmisc.tile([2, bsh], F32, tag="m2")
                nc.tensor.matmul(csum[0:1, :], ones2[:], lp[:],
                                 start=True, stop=True)
                csum_sb = smallpool.tile([1, bsh], F32, tag="csum")
                nc.scalar.activation(csum_sb[:], csum[0:1, :], AF.Copy)
                nc.gpsimd.tensor_sub(den_sb[:], den_sb[:], csum_sb[:])
                nc.gpsimd.tensor_scalar_add(den_sb[:], den_sb[:],
                                            float(s_len) * C0)
                nc.sync.dma_start(den_d.ap(), den_sb[:])
                nc.sync.dma_start(num_d.ap(), numacc[:])

    nc.compile()
    return nc


def _get_nc(n_chunks=16, bsh=BSH):
    key = (n_chunks, bsh)
    if key not in _NC_CACHE:
        _NC_CACHE[key] = build(n_chunks, bsh)
    return _NC_CACHE[key]


def _consts():
    bones = np.zeros((128, 2), dtype=F32_NP)
    bones[0:T, 0] = 1.0
    bones[T:128, 1] = 1.0
    bsel = np.zeros((2, 128), dtype=F32_NP)
    bsel[0, 0:T] = 1.0
    bsel[1, T:128] = 1.0
    return bones.astype(BF16_NP), bsel.astype(BF16_NP)


def make_in_maps(emissions, start_transitions, end_transitions, transitions,
                 tags, ncores=NCORES):
    """Host prep: fold start/end into em, convert to bf16, build the two
    DMA-friendly layouts (em_x for the recurrence, em_m + onehots for the
    numerator), shard over cores."""
    em = np.asarray(emissions, dtype=F32_NP).copy()
    em[:, 0, :] += np.asarray(start_transitions, dtype=F32_NP)
    em[:, -1, :] += np.asarray(end_transitions, dtype=F32_NP)
    em_b = em.astype(BF16_NP)
    b_all, s_len = em.shape[0], em.shape[1]
    n_chunks = s_len // CHUNK
    half = n_chunks // 2
    # em_x[c, row, j, b]: rows 0:64 fwd t of chunk c (s = 128c + j);
    # rows 64:128 bwd t of chunk n_chunks-1-c with j reversed
    # (s = s_len-1 - 128c - j)
    fwd = em_b[:, :half * 128, :].reshape(b_all, half, 128, T)
    fwd = fwd.transpose(1, 3, 2, 0)                    # [c, t, j, b]
    bwd = em_b[:, half * 128:, :].reshape(b_all, half, 128, T)
    bwd = bwd[:, ::-1, ::-1, :].transpose(1, 3, 2, 0)  # [c, t, j, b]
    em_x = np.concatenate([fwd, bwd], axis=1)          # [c, 128, 128, b]
    # em_m[g, s, b, t] (natural order per chunk)
    em_m = em_b.reshape(b_all, n_chunks, 128, T).transpose(1, 2, 0, 3)
    # onehots, same layout as em_m
    tags_i = np.asarray(tags).astype(np.int64).reshape(b_all, s_len)
    onehot = np.zeros((b_all, s_len, T), dtype=BF16_NP)
    np.put_along_axis(onehot, tags_i[:, :, None], BF16_NP(1.0), axis=2)
    oh_m = onehot.reshape(b_all, n_chunks, 128, T).transpose(1, 2, 0, 3)
    trans = np.asarray(transitions, dtype=F32_NP).reshape(T, T)
    blockw = np.zeros((128, 128), dtype=BF16_NP)
    blockw[0:T, 0:T] = np.exp(trans).astype(BF16_NP)
    blockw[T:128, T:128] = np.exp(trans.T).astype(BF16_NP)
    bones, bsel = _consts()
    bsh = b_all // ncores
    in_maps = []
    for cidx in range(ncores):
        sl = slice(cidx * bsh, (cidx + 1) * bsh)
        in_maps.append({
            "emx": np.ascontiguousarray(em_x[:, :, :, sl]),
            "emm": np.ascontiguousarray(em_m[:, :, sl, :]),
            "oh": np.ascontiguousarray(oh_m[:, :, sl, :]),
            "blockw": blockw,
            "bones": bones,
            "bsel": bsel,
        })
    return in_maps


def _host_trans_part(transitions, tags):
    tags_i = np.asarray(tags).astype(np.int64)
    pair_idx = tags_i[:, :-1] * T + tags_i[:, 1:]
    hist = np.bincount(pair_idx.ravel(), minlength=T * T).reshape(T, T)
    trans = np.asarray(transitions, dtype=np.float64).reshape(T, T)
    return float((hist * trans).sum())


def kernel(emissions, start_transitions, end_transitions, transitions,
           tags, mask):
    """Full-input entry point; shards over 8 NeuronCores internally."""
    from concourse.bass_utils import run_bass_kernel_spmd

    emissions = np.asarray(emissions)
    assert emissions.shape == (B, S, T)
    assert (np.asarray(mask) != 0).all(), "kernel assumes all-ones mask"

    in_maps = make_in_maps(emissions, start_transitions, end_transitions,
                           transitions, tags)
    nc = _get_nc()
    res = run_bass_kernel_spmd(nc, in_maps, core_ids=list(range(NCORES)))

    num_total = _host_trans_part(transitions, tags)
    den_total = 0.0
    for cidx in range(NCORES):
        r = res.results[cidx]
        num_total += float(r["numacc"].astype(np.float64).sum())
        den_total += float(r["den"].astype(np.float64).sum())
    loss = -(num_total - den_total) / float(B)
    return np.float32(loss)


# revision 14
# speedup vs baseline: 15.1925x; 3.8451x over previous
"""CRF negative-log-likelihood loss kernel for Trainium2 (8 NeuronCores).

Problem: B=256, S=2048, T=64 CRF loss (torchcrf-style), mask all-ones.

Strategy (v3: blocked chains)
-----------------------------
Data-parallel over batch: each of the 8 cores gets 32 batch rows.

Denominator (log-partition): forward/backward meet-in-the-middle in the
exp domain, PLUS a sequence-blocking trick that exploits the Hilbert-
metric contraction of positive matrix products: each half (1024 steps)
is split into K blocks that run as INDEPENDENT chains.  Block 0 starts
from the true initial state; blocks k>=1 start w rounds early from an
arbitrary positive state (the emission vector at that position) and
"burn in" -- after w warmup steps the state direction has converged to
the true one within ~0.46^w (validated 3e-6 end-to-end), so the
measured per-step mass growth over the block's window is exact for all
practical purposes.  The measured window's growth is recovered as
  ln(final mass) - ln(mass at warmup boundary) - sum ln(renorms),
with the boundary mass folded into the same per-chain accumulator
`pacc` that logs the bf16 renormalization factors (product form).

All 2K blocks (K fwd + K bwd) advance together in ONE round:
a 128x128 block-diagonal matmul (W / W^T) against a [128, K*32] state,
then one DVE multiply with the per-round emission-exp slice.  Serial
rounds drop from 1023 to R = (1024 + (K-1)(w+1)) / K  (~142 at K=8).

Numerator: onehots built on the HOST and DMA'd; per 128-step chunk one
Pool-engine multiply + reduce against em (no PE/DVE involvement).
Transition part sum trans[tag_s, tag_s+1] depends only on tags ->
host bincount in f64.  start/end transitions are folded into em rows
s=0 / s=S-1 on the host.

All log/assembly math runs on the host in f64 from raw f32 dumps
(zall, pacc, zj, numacc) -- the device never calls Ln.
"""

import contextlib

import numpy as np
import ml_dtypes

F32_NP = np.float32
BF16_NP = ml_dtypes.bfloat16

B, S, T = 256, 2048, 64
NCORES = 8
BSH = B // NCORES  # 32
C0 = 4.8204  # ~ ln(64 * e^0.5 * sinh(1)) : expected per-step log growth
RN = 64      # renorm every RN rounds
KBLK = 8     # blocks per direction
WUP = 15     # warmup rounds for blocks >= 1

_NC_CACHE = {}


def _plan(kblk=KBLK, wup=WUP):
    half = S // 2
    R = (half + (kblk - 1) * (wup + 1)) // kblk
    assert R * kblk - (kblk - 1) * (wup + 1) == half, (kblk, wup)
    cols = 2 * T * kblk * BSH // 128  # = kblk*BSH (128-part state)
    ch = max(8, min(32, 8192 // cols))  # x-chunk rounds
    n_xch = -(-R // ch)
    return R, cols, ch, n_xch


def build(nrep=1, fake_x=False, no_num=False, no_rounds=False,
          rn=RN, kblk=KBLK, wup=WUP, pround_bufs=4, spool_bufs=6,
          warm=0, no_ldw_wait=False, num_dve=False, bsh=BSH):
    """Build + compile the per-core Bass module."""
    import concourse.bacc as bacc
    import concourse.mybir as mybir
    import concourse.tile as tile

    F32 = mybir.dt.float32
    BF16 = mybir.dt.bfloat16
    AF = mybir.ActivationFunctionType
    ALU = mybir.AluOpType

    R, cols, CH, n_xch = _plan(kblk, wup)
    n_chunks = S // 128  # numerator chunks

    nc = bacc.Bacc("TRN2", target_bir_lowering=False, debug=False,
                   num_devices=NCORES)
    if no_ldw_wait:
        nc.move_matmul_waits_to_ldweights = lambda: None

    em_x_d = nc.dram_tensor("emx", [n_xch, 128, CH, cols], BF16,
                            kind="ExternalInput")
    em_m_d = nc.dram_tensor("emm", [n_chunks, 128, bsh, T], BF16,
                            kind="ExternalInput")
    oh_d = nc.dram_tensor("oh", [n_chunks, 128, bsh, T], BF16,
                          kind="ExternalInput")
    blockw_d = nc.dram_tensor("blockw", [128, 128], BF16,
                              kind="ExternalInput")
    bones_d = nc.dram_tensor("bones", [128, 2], BF16, kind="ExternalInput")
    bsel_d = nc.dram_tensor("bsel", [2, 128], BF16, kind="ExternalInput")
    mask0_d = nc.dram_tensor("mask0", [2, cols], F32, kind="ExternalInput")
    mask1_d = nc.dram_tensor("mask1", [2, cols], F32, kind="ExternalInput")
    zall_d = nc.dram_tensor("zall", [2, cols], F32, kind="ExternalOutput")
    pacc_d = nc.dram_tensor("pacco", [2, cols], F32, kind="ExternalOutput")
    zj_d = nc.dram_tensor("zj", [1, bsh], F32, kind="ExternalOutput")
    num_d = nc.dram_tensor("numacc", [128, n_chunks], F32,
                           kind="ExternalOutput")

    with tile.TileContext(nc) as tc, nc.allow_low_precision(
            reason="bf16 state/weights validated against f64 reference"):
        with (
            tc.tile_pool(name="consts", bufs=1) as consts,
            tc.tile_pool(name="xchunk", bufs=2) as xpool,
            tc.tile_pool(name="xraw", bufs=2) as xrawpool,
            tc.tile_pool(name="emt", bufs=4) as empool,
            tc.tile_pool(name="oht", bufs=4) as ohpool,
            tc.tile_pool(name="scr", bufs=2) as scrpool,
            tc.tile_pool(name="state", bufs=spool_bufs) as spool,
            tc.tile_pool(name="rm", bufs=3) as rmpool,
            tc.tile_pool(name="small", bufs=4) as smallpool,
            tc.tile_pool(name="pround", bufs=pround_bufs,
                         space="PSUM") as pround,
            tc.tile_pool(name="pmisc", bufs=2, space="PSUM") as pmisc,
        ):
            rep_ctx = (tc.For_i(0, nrep, 1) if nrep > 1
                       else contextlib.nullcontext())
            with rep_ctx:
                # ---------------- constants / setup ----------------
                blockw = consts.tile([128, 128], BF16, tag="blockw")
                nc.sync.dma_start(blockw[:], blockw_d.ap())
                blockones = consts.tile([128, 2], BF16, tag="blockones")
                nc.sync.dma_start(blockones[:], bones_d.ap())
                blocksel = consts.tile([2, 128], BF16, tag="blocksel")
                nc.sync.dma_start(blocksel[:], bsel_d.ap())
                mask0 = consts.tile([2, cols], F32, tag="mask0")
                nc.sync.dma_start(mask0[:], mask0_d.ap())
                mask1 = consts.tile([2, cols], F32, tag="mask1")
                nc.sync.dma_start(mask1[:], mask1_d.ap())
                ones64 = consts.tile([T, 1], F32, tag="ones64")
                nc.vector.memset(ones64[:], 1.0)
                negc0 = consts.tile([128, 1], F32, tag="negc0")
                nc.vector.memset(negc0[:], -C0)

                pacc = consts.tile([2, cols], F32, tag="pacc")
                nc.vector.memset(pacc[:], 1.0)
                numacc = consts.tile([128, n_chunks], F32, tag="numacc")
                nc.vector.memset(numacc[:], 0.0)

                emg = {}      # numerator em chunk g
                ohg = {}      # numerator onehot chunk g
                exraw = {}    # x chunk c raw (pre-exp)

                def dma_x(c):
                    xr = xrawpool.tile([128, CH, cols], BF16, tag="xr")
                    nc.sync.dma_start(xr[:], em_x_d.ap()[c])
                    exraw[c] = xr

                def dma_num(g):
                    eg = empool.tile([128, bsh, T], BF16, tag="em")
                    nc.sync.dma_start(eg[:], em_m_d.ap()[g])
                    emg[g] = eg
                    og = ohpool.tile([128, bsh, T], BF16, tag="oh")
                    nc.sync.dma_start(og[:], oh_d.ap()[g])
                    ohg[g] = og

                def num_chunk(g):
                    """Gather-reduce (em*onehot) for chunk g, off-engines."""
                    if num_dve:
                        scr = scrpool.tile([128, bsh, T], BF16, tag="scr")
                        nc.vector.scalar_tensor_tensor(
                            scr[:], emg[g][:], 1.0, ohg[g][:],
                            op0=ALU.bypass, op1=ALU.mult,
                            accum_out=numacc[:, g:g + 1])
                    else:
                        scr = scrpool.tile([128, bsh, T], BF16, tag="scr")
                        nc.gpsimd.tensor_tensor(
                            scr[:], emg[g][:], ohg[g][:], op=ALU.mult)
                        nc.gpsimd.tensor_reduce(
                            numacc[0:1, g:g + 1], scr[:],
                            axis=mybir.AxisListType.XYZWC, op=ALU.add)
                    del emg[g], ohg[g]

                def x_quanta(c, nrounds):
                    """ACT-exp quanta producing X chunk c from em_x."""
                    xc = xpool.tile([128, CH, cols], BF16, tag="xc")
                    if fake_x:
                        def q():
                            nc.gpsimd.memset(xc[:], 0.0133)
                        return xc, [q]
                    qs = []
                    qn = 4
                    step = -(-nrounds // qn)
                    for hj in range(qn):
                        lo = hj * step
                        hi = min(nrounds, lo + step)
                        if lo >= hi:
                            continue
                        def q(lo=lo, hi=hi, last=(hi >= nrounds)):
                            nc.scalar.activation(
                                xc[:, lo:hi, :], exraw[c][:, lo:hi, :],
                                AF.Exp, bias=negc0[:])
                            if last:
                                del exraw[c]
                        qs.append(q)
                    return xc, qs

                # ---------------- priming ----------------
                from collections import deque
                bg = deque()
                xchunks = {}
                if not fake_x:
                    dma_x(0)
                    if n_xch > 1:
                        dma_x(1)
                    if not no_num:
                        for g in range(min(3, n_chunks)):
                            dma_num(g)
                for c in (0, 1):
                    if c <= n_xch - 1:
                        nr = min(CH, R - c * CH)
                        xc, qs = x_quanta(c, nr)
                        [q() for q in qs]
                        xchunks[c] = xc

                state = spool.tile([128, cols], BF16, tag="st")
                nc.vector.tensor_copy(state[:], xchunks[0][:, 0, :])

                if warm:
                    wsrc = consts.tile([128, bsh], BF16, tag="wsrc")
                    nc.vector.memset(wsrc[:], 0.5)

                # numerator schedule: chunk g at round 2 + step*g
                r_end = 0 if no_rounds else R - 1
                num_step = max(1, (r_end - 4) // max(1, n_chunks))

                def num_slot(r):
                    if no_num or fake_x or r < 2 or (r - 2) % num_step:
                        return None
                    g = (r - 2) // num_step
                    return g if g < n_chunks else None

                for r in range(1, r_end + 1):
                    c, j = divmod(r, CH)
                    if (j == 1 and not fake_x and c + 1 <= n_xch - 1
                            and (c + 1) not in exraw
                            and (c + 1) not in xchunks):
                        dma_x(c + 1)
                    if j == 2 and c + 1 <= n_xch - 1 and \
                            (c + 1) not in xchunks:
                        nr = min(CH, R - (c + 1) * CH)
                        xc, qs = x_quanta(c + 1, nr)
                        xchunks[c + 1] = xc
                        bg.extend(qs)
                        xchunks.pop(c - 1, None)
                    g = num_slot(r)
                    if g is not None:
                        num_chunk(g)
                        if g + 3 < n_chunks:
                            dma_num(g + 3)
                    if bg:
                        bg.popleft()()
                    for _ in range(warm):
                        wps = pround.tile([128, bsh], F32, tag="wp")
                        nc.tensor.matmul(wps[:], blockw[:], wsrc[:],
                                         start=True, stop=True,
                                         skip_group_check=True)
                    p = pround.tile([128, cols], F32, tag="p")
                    nc.tensor.matmul(p[:], blockw[:], state[:],
                                     start=True, stop=True)
                    nstate = spool.tile([128, cols], BF16, tag="st")
                    nc.vector.tensor_mul(nstate[:], p[:],
                                         xchunks[c][:, j, :])
                    state = nstate

                    if r == wup:
                        # warmup boundary: pacc := pacc*mask0 + mass*mask1
                        mass = pmisc.tile([2, cols], F32, tag="m2")
                        nc.tensor.matmul(mass[:], blockones[:], state[:],
                                         start=True, stop=True)
                        nc.vector.tensor_mul(pacc[:], pacc[:], mask0[:])
                        bmt = smallpool.tile([2, cols], F32, tag="bmt")
                        nc.vector.scalar_tensor_tensor(
                            bmt[:], mass[:], 1.0, mask1[:],
                            op0=ALU.bypass, op1=ALU.mult)
                        nc.vector.tensor_add(pacc[:], pacc[:], bmt[:])

                    if r % rn == 0 and r < r_end:
                        mass = pmisc.tile([2, cols], F32, tag="m2")
                        nc.tensor.matmul(mass[:], blockones[:], state[:],
                                         start=True, stop=True)
                        rmass = rmpool.tile([2, cols], BF16, tag="rm")
                        nc.vector.reciprocal(rmass[:], mass[:])
                        nc.vector.tensor_mul(pacc[:], pacc[:], rmass[:])
                        rbc = pmisc.tile([128, cols], F32, tag="rbc")
                        nc.tensor.matmul(rbc[:], blocksel[:], rmass[:],
                                         start=True, stop=True)
                        nstate = spool.tile([128, cols], BF16, tag="st")
                        nc.vector.tensor_mul(nstate[:], state[:], rbc[:])
                        state = nstate

                while bg:
                    bg.popleft()()
                if no_rounds and not fake_x and not no_num:
                    for g in range(3, n_chunks):
                        dma_num(g)
                    for g in range(n_chunks):
                        num_chunk(g)

                # ---------------- final reads ----------------
                zps = pmisc.tile([2, cols], F32, tag="m2")
                nc.tensor.matmul(zps[:], blockones[:], state[:],
                                 start=True, stop=True)
                zall_sb = smallpool.tile([2, cols], F32, tag="zall")
                nc.vector.tensor_copy(zall_sb[:], zps[:])
                nc.sync.dma_start(zall_d.ap(), zall_sb[:])
                nc.sync.dma_start(pacc_d.ap(), pacc[:])

                lastc = slice((kblk - 1) * bsh, kblk * bsh)
                pf = pround.tile([128, cols], F32, tag="p")
                nc.tensor.matmul(pf[0:T, 0:bsh], blockw[T:128, T:128],
                                 state[T:128, lastc], start=True, stop=True)
                y = smallpool.tile([T, bsh], F32, tag="y")
                nc.vector.tensor_mul(y[:], state[0:T, lastc], pf[0:T, 0:bsh])
                zjp = pmisc.tile([2, cols], F32, tag="m2")
                nc.tensor.matmul(zjp[0:1, 0:bsh], ones64[:], y[:],
                                 start=True, stop=True)
                zj_sb = smallpool.tile([1, bsh], F32, tag="zj")
                nc.vector.tensor_copy(zj_sb[:], zjp[0:1, 0:bsh])
                nc.sync.dma_start(zj_d.ap(), zj_sb[:])
                nc.sync.dma_start(num_d.ap(), numacc[:])

    nc.compile()
    return nc


def _get_nc():
    key = "v3"
    if key not in _NC_CACHE:
        _NC_CACHE[key] = build()
    return _NC_CACHE[key]


def _consts():
    bones = np.zeros((128, 2), dtype=F32_NP)
    bones[0:T, 0] = 1.0
    bones[T:128, 1] = 1.0
    bsel = np.zeros((2, 128), dtype=F32_NP)
    bsel[0, 0:T] = 1.0
    bsel[1, T:128] = 1.0
    return bones.astype(BF16_NP), bsel.astype(BF16_NP)


def make_in_maps(emissions, start_transitions, end_transitions, transitions,
                 tags, ncores=NCORES, kblk=KBLK, wup=WUP):
    """Host prep: fold start/end into em, convert to bf16, build the
    blocked x layout + numerator layouts, shard over cores."""
    R, cols, CH, n_xch = _plan(kblk, wup)
    em = np.asarray(emissions, dtype=F32_NP).copy()
    em[:, 0, :] += np.asarray(start_transitions, dtype=F32_NP)
    em[:, -1, :] += np.asarray(end_transitions, dtype=F32_NP)
    em_b = em.astype(BF16_NP)
    b_all, s_len = em.shape[0], em.shape[1]
    n_chunks = s_len // 128
    L = R - 1 - wup
    # positions per (k, round j)
    P = np.empty((kblk, R), dtype=np.int64)
    P[0] = np.arange(R)
    for k in range(1, kblk):
        a_k = R + (k - 1) * L
        P[k] = a_k - wup - 1 + np.arange(R)
    fx = em_b[:, P, :]                    # [B, K, R, T]
    bx = em_b[:, s_len - 1 - P, :]        # [B, K, R, T]
    X = np.concatenate([fx, bx], axis=3)  # [B, K, R, 2T]
    Rpad = n_xch * CH
    if Rpad > R:
        pad = np.zeros((b_all, kblk, Rpad - R, 2 * T), dtype=BF16_NP)
        X = np.concatenate([X, pad], axis=2)
    # -> [c, t(128), j2, k, b]
    X = X.reshape(b_all, kblk, n_xch, CH, 2 * T)
    X = X.transpose(2, 4, 3, 1, 0)        # [c, 128, CH, K, B]
    # em_m[g, s, b, t] + onehots
    em_m = em_b.reshape(b_all, n_chunks, 128, T).transpose(1, 2, 0, 3)
    tags_i = np.asarray(tags).astype(np.int64).reshape(b_all, s_len)
    onehot = np.zeros((b_all, s_len, T), dtype=BF16_NP)
    np.put_along_axis(onehot, tags_i[:, :, None], BF16_NP(1.0), axis=2)
    oh_m = onehot.reshape(b_all, n_chunks, 128, T).transpose(1, 2, 0, 3)
    trans = np.asarray(transitions, dtype=F32_NP).reshape(T, T)
    blockw = np.zeros((128, 128), dtype=BF16_NP)
    blockw[0:T, 0:T] = np.exp(trans).astype(BF16_NP)
    blockw[T:128, T:128] = np.exp(trans.T).astype(BF16_NP)
    bones, bsel = _consts()
    bsh = b_all // ncores
    m0 = np.zeros((2, kblk, bsh), dtype=F32_NP)
    m0[:, 0, :] = 1.0
    m0 = m0.reshape(2, kblk * bsh)
    m1 = (1.0 - m0).astype(F32_NP)
    in_maps = []
    for cidx in range(ncores):
        sl = slice(cidx * bsh, (cidx + 1) * bsh)
        Xc = X[:, :, :, :, sl].reshape(n_xch, 128, CH, kblk * bsh)
        in_maps.append({
            "emx": np.ascontiguousarray(Xc),
            "emm": np.ascontiguousarray(em_m[:, :, sl, :]),
            "oh": np.ascontiguousarray(oh_m[:, :, sl, :]),
            "blockw": blockw,
            "bones": bones,
            "bsel": bsel,
            "mask0": m0,
            "mask1": m1,
        })
    return in_maps


def _host_trans_part(transitions, tags):
    tags_i = np.asarray(tags).astype(np.int64)
    pair_idx = tags_i[:, :-1] * T + tags_i[:, 1:]
    hist = np.bincount(pair_idx.ravel(), minlength=T * T).reshape(T, T)
    trans = np.asarray(transitions, dtype=np.float64).reshape(T, T)
    return float((hist * trans).sum())


def kernel(emissions, start_transitions, end_transitions, transitions,
           tags, mask):
    """Full-input entry point; shards over 8 NeuronCores internally."""
    from concourse.bass_utils import run_bass_kernel_spmd

    emissions = np.asarray(emissions)
    assert emissions.shape == (B, S, T)
    assert (np.asarray(mask) != 0).all(), "kernel assumes all-ones mask"

    in_maps = make_in_maps(emissions, start_transitions, end_transitions,
                           transitions, tags)
    nc = _get_nc()
    res = run_bass_kernel_spmd(nc, in_maps, core_ids=list(range(NCORES)))

    kblk, bsh = KBLK, BSH
    num_total = _host_trans_part(transitions, tags)
    den_total = 0.0
    for cidx in range(NCORES):
        r = res.results[cidx]
        num_total += float(r["numacc"].astype(np.float64).sum())
        zall = r["zall"].astype(np.float64).reshape(2, kblk, bsh)
        pacc = r["pacco"].astype(np.float64).reshape(2, kblk, bsh)
        zj = r["zj"].astype(np.float64).reshape(bsh)
        den = np.zeros(bsh, dtype=np.float64)
        for row in range(2):
            for k in range(kblk - 1):
                den += np.log(zall[row, k]) - np.log(pacc[row, k])
            den += -np.log(pacc[row, kblk - 1])
        den += np.log(zj) + float(S) * C0
        den_total += float(den.sum())
    loss = -(num_total - den_total) / float(B)
    return np.float32(loss)


# revision 41
# speedup vs baseline: 41.0695x; 2.7033x over previous
"""CRF negative-log-likelihood loss kernel for Trainium2 (8 NeuronCores).

Problem: B=256, S=2048, T=64 CRF loss (torchcrf-style), mask all-ones.

Strategy (v3: blocked chains)
-----------------------------
Data-parallel over batch: each of the 8 cores gets 32 batch rows.

Denominator (log-partition): forward/backward meet-in-the-middle in the
exp domain, PLUS a sequence-blocking trick that exploits the Hilbert-
metric contraction of positive matrix products: each half (1024 steps)
is split into K blocks that run as INDEPENDENT chains.  Block 0 starts
from the true initial state; blocks k>=1 start w rounds early from an
arbitrary positive state (the emission vector at that position) and
"burn in" -- after w warmup steps the state direction has converged to
the true one within ~0.46^w (validated 3e-6 end-to-end), so the
measured per-step mass growth over the block's window is exact for all
practical purposes.  The measured window's growth is recovered as
  ln(final mass) - ln(mass at warmup boundary) - sum ln(renorms),
with the boundary mass folded into the same per-chain accumulator
`pacc` that logs the bf16 renormalization factors (product form).

All 2K blocks (K fwd + K bwd) advance together in ONE round:
a 128x128 block-diagonal matmul (W / W^T) against a [128, K*32] state,
then one DVE multiply with the per-round emission-exp slice.  Serial
rounds drop from 1023 to R = (1024 + (K-1)(w+1)) / K  (~142 at K=8).

Numerator: onehots built on the HOST and DMA'd; per 128-step chunk one
Pool-engine multiply + reduce against em (no PE/DVE involvement).
Transition part sum trans[tag_s, tag_s+1] depends only on tags ->
host bincount in f64.  start/end transitions are folded into em rows
s=0 / s=S-1 on the host.

All log/assembly math runs on the host in f64 from raw f32 dumps
(zall, pacc, zj, numacc) -- the device never calls Ln.
"""

import contextlib
from collections import deque

import numpy as np
import ml_dtypes

F32_NP = np.float32
BF16_NP = ml_dtypes.bfloat16

B, S, T = 256, 2048, 64
NCORES = 8
BSH = B // NCORES  # 32
C0 = 4.8204  # ~ ln(64 * e^0.5 * sinh(1)) : expected per-step log growth
RN = 128     # renorm every RN rounds (R=79 -> no renorm needed at all)
KBLK = 16    # blocks per direction
WUP = 15     # warmup rounds for blocks >= 1

_NC_CACHE = {}


def _plan(kblk=KBLK, wup=WUP):
    half = S // 2
    R = (half + (kblk - 1) * (wup + 1)) // kblk
    assert R * kblk - (kblk - 1) * (wup + 1) == half, (kblk, wup)
    cols = 2 * T * kblk * BSH // 128  # = kblk*BSH (128-part state)
    ch = max(8, min(32, 8192 // cols))  # x-chunk rounds
    n_xch = -(-R // ch)
    return R, cols, ch, n_xch


def build(nrep=1, fake_x=False, no_num=False, no_rounds=False,
          rn=RN, kblk=KBLK, wup=WUP, pround_bufs=4, spool_bufs=6,
          warm=0, no_ldw_wait=True, num_dve=False, num_pe=True,
          dmasplit=4, bsh=BSH):
    """Build + compile the per-core Bass module."""
    import concourse.bacc as bacc
    import concourse.mybir as mybir
    import concourse.tile as tile

    F32 = mybir.dt.float32
    BF16 = mybir.dt.bfloat16
    AF = mybir.ActivationFunctionType
    ALU = mybir.AluOpType

    R, cols, CH, n_xch = _plan(kblk, wup)
    n_chunks = S // 128  # numerator chunks

    nc = bacc.Bacc("TRN2", target_bir_lowering=False, debug=False,
                   num_devices=NCORES)
    if no_ldw_wait:
        nc.move_matmul_waits_to_ldweights = lambda: None

    em_x_d = nc.dram_tensor("emx", [n_xch, 128, CH, cols], BF16,
                            kind="ExternalInput")
    em_m_d = nc.dram_tensor("emm", [n_chunks, 128, bsh, T], BF16,
                            kind="ExternalInput")
    oh_d = nc.dram_tensor("oh", [n_chunks, 128, bsh, T], BF16,
                          kind="ExternalInput")
    blockw_d = nc.dram_tensor("blockw", [128, 128], BF16,
                              kind="ExternalInput")
    bones_d = nc.dram_tensor("bones", [128, 2], BF16, kind="ExternalInput")
    bsel_d = nc.dram_tensor("bsel", [2, 128], BF16, kind="ExternalInput")
    mask0_d = nc.dram_tensor("mask0", [2, cols], F32, kind="ExternalInput")
    mask1_d = nc.dram_tensor("mask1", [2, cols], F32, kind="ExternalInput")
    zall_d = nc.dram_tensor("zall", [2, cols], F32, kind="ExternalOutput")
    pacc_d = nc.dram_tensor("pacco", [2, cols], F32, kind="ExternalOutput")
    zj_d = nc.dram_tensor("zj", [1, bsh], F32, kind="ExternalOutput")
    NQ = 8  # numerator sub-quanta per chunk (num_dve mode)
    if num_pe:
        num_d = nc.dram_tensor("numacc", [T, T], F32,
                               kind="ExternalOutput")
    else:
        num_cols = n_chunks * NQ if num_dve else n_chunks
        num_d = nc.dram_tensor("numacc", [128, num_cols], F32,
                               kind="ExternalOutput")

    with tile.TileContext(nc) as tc, nc.allow_low_precision(
            reason="bf16 state/weights validated against f64 reference"):
        with (
            tc.tile_pool(name="consts", bufs=1) as consts,
            tc.tile_pool(name="xchunk", bufs=2) as xpool,
            tc.tile_pool(name="xraw", bufs=2) as xrawpool,
            tc.tile_pool(name="emt", bufs=4) as empool,
            tc.tile_pool(name="oht", bufs=4) as ohpool,
            tc.tile_pool(name="scr", bufs=2) as scrpool,
            tc.tile_pool(name="state", bufs=spool_bufs) as spool,
            tc.tile_pool(name="rm", bufs=3) as rmpool,
            tc.tile_pool(name="small", bufs=4) as smallpool,
            tc.tile_pool(name="pround", bufs=pround_bufs,
                         space="PSUM") as pround,
            tc.tile_pool(name="pmisc", bufs=1, space="PSUM") as pmisc,
            tc.tile_pool(name="pnum", bufs=1, space="PSUM") as pnum,
        ):
            rep_ctx = (tc.For_i(0, nrep, 1) if nrep > 1
                       else contextlib.nullcontext())
            with rep_ctx:
                # ---------------- constants / setup ----------------
                blockw = consts.tile([128, 128], BF16, tag="blockw")
                nc.sync.dma_start(blockw[:], blockw_d.ap())
                blockones = consts.tile([128, 2], BF16, tag="blockones")
                nc.sync.dma_start(blockones[:], bones_d.ap())
                blocksel = consts.tile([2, 128], BF16, tag="blocksel")
                nc.sync.dma_start(blocksel[:], bsel_d.ap())
                mask0 = consts.tile([2, cols], F32, tag="mask0")
                nc.sync.dma_start(mask0[:], mask0_d.ap())
                mask1 = consts.tile([2, cols], F32, tag="mask1")
                nc.sync.dma_start(mask1[:], mask1_d.ap())
                ones64 = consts.tile([T, 1], F32, tag="ones64")
                nc.vector.memset(ones64[:], 1.0)
                negc0 = consts.tile([128, 1], F32, tag="negc0")
                nc.vector.memset(negc0[:], -C0)

                pacc = consts.tile([2, cols], F32, tag="pacc")
                nc.vector.memset(pacc[:], 1.0)
                if num_pe:
                    m_ps = pnum.tile([T, T], F32, tag="mps")
                    num_state = {"first": True, "last": None}
                else:
                    numacc = consts.tile([128, num_cols], F32,
                                         tag="numacc")
                    nc.vector.memset(numacc[:], 0.0)

                emg = {}      # numerator em chunk g
                ohg = {}      # numerator onehot chunk g
                exraw = {}    # x chunk c raw (pre-exp)

                def dma_x(c):
                    xr = xrawpool.tile([128, CH, cols], BF16, tag="xr")
                    step = -(-CH // dmasplit)
                    for lo in range(0, CH, step):
                        hi = min(CH, lo + step)
                        nc.sync.dma_start(xr[:, lo:hi, :],
                                          em_x_d.ap()[c, :, lo:hi, :])
                    exraw[c] = xr

                def dma_num(g):
                    eg = empool.tile([128, bsh, T], BF16, tag="em")
                    nc.sync.dma_start(eg[:], em_m_d.ap()[g])
                    emg[g] = eg
                    og = ohpool.tile([128, bsh, T], BF16, tag="oh")
                    nc.sync.dma_start(og[:], oh_d.ap()[g])
                    ohg[g] = og

                bgn = deque()  # numerator sub-quanta (num_dve mode)
                bgm = deque()  # numerator matmuls (num_pe mode)

                def num_chunk(g):
                    """Gather-reduce (em*onehot) for chunk g, off-chain."""
                    if num_pe:
                        # one accumulating PE matmul per batch row:
                        # m_ps += onehot_{g,b}^T @ em_{g,b}
                        for b in range(bsh):
                            def q(g=g, b=b, last=(b == bsh - 1)):
                                mm = nc.tensor.matmul(
                                    m_ps[:], ohg[g][:, b, :],
                                    emg[g][:, b, :],
                                    start=num_state["first"], stop=False,
                                    skip_group_check=True)
                                num_state["first"] = False
                                num_state["last"] = mm
                                if last:
                                    del emg[g], ohg[g]
                            bgm.append(q)
                        return
                    use_dve = (num_dve is True) or \
                        (num_dve == "mixed" and g % 2 == 1)
                    if use_dve:
                        # 8 DVE sub-quanta, popped one per round (small
                        # enough to hide in the chain's DVE idle slack)
                        bq = bsh // NQ
                        for qi in range(NQ):
                            sl = slice(qi * bq, (qi + 1) * bq)
                            def q(g=g, qi=qi, sl=sl,
                                  last=(qi == NQ - 1)):
                                scr = scrpool.tile([128, bq, T], BF16,
                                                   tag="scr")
                                nc.vector.scalar_tensor_tensor(
                                    scr[:], emg[g][:, sl, :], 1.0,
                                    ohg[g][:, sl, :],
                                    op0=ALU.bypass, op1=ALU.mult,
                                    accum_out=numacc[:, g * NQ + qi:
                                                     g * NQ + qi + 1])
                                if last:
                                    del emg[g], ohg[g]
                            bgn.append(q)
                    else:
                        gc = g * NQ if num_dve else g
                        scr = scrpool.tile([128, bsh, T], BF16, tag="scr")
                        nc.gpsimd.tensor_tensor(
                            scr[:], emg[g][:], ohg[g][:], op=ALU.mult)
                        nc.gpsimd.tensor_reduce(
                            numacc[0:1, gc:gc + 1], scr[:],
                            axis=mybir.AxisListType.XYZWC, op=ALU.add)
                        del emg[g], ohg[g]

                def x_quanta(c, nrounds):
                    """ACT-exp quanta producing X chunk c from em_x."""
                    xc = xpool.tile([128, CH, cols], BF16, tag="xc")
                    if fake_x:
                        def q():
                            nc.gpsimd.memset(xc[:], 0.0133)
                        return xc, [q]
                    qs = []
                    qn = 4
                    step = -(-nrounds // qn)
                    for hj in range(qn):
                        lo = hj * step
                        hi = min(nrounds, lo + step)
                        if lo >= hi:
                            continue
                        def q(lo=lo, hi=hi, last=(hi >= nrounds)):
                            nc.scalar.activation(
                                xc[:, lo:hi, :], exraw[c][:, lo:hi, :],
                                AF.Exp, bias=negc0[:])
                            if last:
                                del exraw[c]
                        qs.append(q)
                    return xc, qs

                # ---------------- priming ----------------
                bg = deque()
                xchunks = {}
                if not fake_x:
                    dma_x(0)
                    if n_xch > 1:
                        dma_x(1)
                    if not no_num:
                        for g in range(min(3, n_chunks)):
                            dma_num(g)
                for c in (0, 1):
                    if c <= n_xch - 1:
                        nr = min(CH, R - c * CH)
                        xc, qs = x_quanta(c, nr)
                        [q() for q in qs]
                        xchunks[c] = xc

                state = spool.tile([128, cols], BF16, tag="st")
                nc.vector.tensor_copy(state[:], xchunks[0][:, 0, :])

                if warm:
                    wsrc = consts.tile([128, bsh], BF16, tag="wsrc")
                    nc.vector.memset(wsrc[:], 0.5)

                # numerator schedule: chunk g at round 2 + step*g
                r_end = 0 if no_rounds else R - 1
                num_step = max(1, (r_end - 4) // max(1, n_chunks))

                def num_slot(r):
                    if no_num or fake_x or r < 2 or (r - 2) % num_step:
                        return None
                    g = (r - 2) // num_step
                    return g if g < n_chunks else None

                for r in range(1, r_end + 1):
                    c, j = divmod(r, CH)
                    if (j == 1 and not fake_x and c + 1 <= n_xch - 1
                            and (c + 1) not in exraw
                            and (c + 1) not in xchunks):
                        dma_x(c + 1)
                    if j == 2 and c + 1 <= n_xch - 1 and \
                            (c + 1) not in xchunks:
                        nr = min(CH, R - (c + 1) * CH)
                        xc, qs = x_quanta(c + 1, nr)
                        xchunks[c + 1] = xc
                        bg.extend(qs)
                        xchunks.pop(c - 1, None)
                    g = num_slot(r)
                    if g is not None:
                        num_chunk(g)
                        if g + 3 < n_chunks:
                            dma_num(g + 3)
                    if bg:
                        bg.popleft()()
                    if bgn:
                        bgn.popleft()()
                    for _ in range(7):
                        if not bgm:
                            break
                        bgm.popleft()()
                    for _ in range(warm):
                        wps = pround.tile([128, cols], F32, tag="p")
                        nc.tensor.matmul(wps[:, 0:bsh], blockw[:], wsrc[:],
                                         start=True, stop=True,
                                         skip_group_check=True)
                    p = pround.tile([128, cols], F32, tag="p")
                    nc.tensor.matmul(p[:], blockw[:], state[:],
                                     start=True, stop=True)
                    nstate = spool.tile([128, cols], BF16, tag="st")
                    nc.vector.tensor_mul(nstate[:], p[:],
                                         xchunks[c][:, j, :])
                    state = nstate

                    if r == wup:
                        # warmup boundary: pacc := pacc*mask0 + mass*mask1
                        mass = pmisc.tile([2, cols], F32, tag="m2")
                        nc.tensor.matmul(mass[:], blockones[:], state[:],
                                         start=True, stop=True)
                        nc.vector.tensor_mul(pacc[:], pacc[:], mask0[:])
                        bmt = smallpool.tile([2, cols], F32, tag="bmt")
                        nc.vector.scalar_tensor_tensor(
                            bmt[:], mass[:], 1.0, mask1[:],
                            op0=ALU.bypass, op1=ALU.mult)
                        nc.vector.tensor_add(pacc[:], pacc[:], bmt[:])

                    if r % rn == 0 and r < r_end:
                        mass = pmisc.tile([2, cols], F32, tag="m2")
                        nc.tensor.matmul(mass[:], blockones[:], state[:],
                                         start=True, stop=True)
                        rmass = rmpool.tile([2, cols], BF16, tag="rm")
                        nc.vector.reciprocal(rmass[:], mass[:])
                        nc.vector.tensor_mul(pacc[:], pacc[:], rmass[:])
                        rbc = pmisc.tile([128, cols], F32, tag="rbc")
                        nc.tensor.matmul(rbc[:], blocksel[:], rmass[:],
                                         start=True, stop=True)
                        nstate = spool.tile([128, cols], BF16, tag="st")
                        nc.vector.tensor_mul(nstate[:], state[:], rbc[:])
                        state = nstate

                while bg:
                    bg.popleft()()
                while bgn:
                    bgn.popleft()()
                while bgm:
                    bgm.popleft()()
                if no_rounds and not fake_x and not no_num:
                    for g in range(3, n_chunks):
                        dma_num(g)
                    for g in range(n_chunks):
                        num_chunk(g)
                    while bgn:
                        bgn.popleft()()
                    while bgm:
                        bgm.popleft()()

                # ---------------- final reads ----------------
                zps = pmisc.tile([2, cols], F32, tag="m2")
                nc.tensor.matmul(zps[:], blockones[:], state[:],
                                 start=True, stop=True)
                zall_sb = smallpool.tile([2, cols], F32, tag="zall")
                nc.vector.tensor_copy(zall_sb[:], zps[:])
                nc.sync.dma_start(zall_d.ap(), zall_sb[:])
                nc.sync.dma_start(pacc_d.ap(), pacc[:])

                lastc = slice((kblk - 1) * bsh, kblk * bsh)
                pf = pround.tile([128, cols], F32, tag="p")
                nc.tensor.matmul(pf[0:T, 0:bsh], blockw[T:128, T:128],
                                 state[T:128, lastc], start=True, stop=True)
                y = smallpool.tile([T, bsh], F32, tag="y")
                nc.vector.tensor_mul(y[:], state[0:T, lastc], pf[0:T, 0:bsh])
                zjp = pmisc.tile([2, cols], F32, tag="m2")
                nc.tensor.matmul(zjp[0:1, 0:bsh], ones64[:], y[:],
                                 start=True, stop=True)
                zj_sb = smallpool.tile([1, bsh], F32, tag="zj")
                nc.vector.tensor_copy(zj_sb[:], zjp[0:1, 0:bsh])
                nc.sync.dma_start(zj_d.ap(), zj_sb[:])
                if num_pe:
                    if num_state["last"] is not None:
                        num_state["last"].ins.stop_tensor_calc = True
                        mps_sb = smallpool.tile([T, T], F32, tag="mpssb")
                        nc.vector.tensor_copy(mps_sb[:], m_ps[:])
                        nc.sync.dma_start(num_d.ap(), mps_sb[:])
                    else:
                        mps_sb = smallpool.tile([T, T], F32, tag="mpssb")
                        nc.vector.memset(mps_sb[:], 0.0)
                        nc.sync.dma_start(num_d.ap(), mps_sb[:])
                else:
                    nc.sync.dma_start(num_d.ap(), numacc[:])

    nc.compile()
    return nc


def _get_nc():
    key = "v3"
    if key not in _NC_CACHE:
        _NC_CACHE[key] = build()
    return _NC_CACHE[key]


def _consts():
    bones = np.zeros((128, 2), dtype=F32_NP)
    bones[0:T, 0] = 1.0
    bones[T:128, 1] = 1.0
    bsel = np.zeros((2, 128), dtype=F32_NP)
    bsel[0, 0:T] = 1.0
    bsel[1, T:128] = 1.0
    return bones.astype(BF16_NP), bsel.astype(BF16_NP)


def make_in_maps(emissions, start_transitions, end_transitions, transitions,
                 tags, ncores=NCORES, kblk=KBLK, wup=WUP):
    """Host prep: fold start/end into em, convert to bf16, build the
    blocked x layout + numerator layouts, shard over cores."""
    R, cols, CH, n_xch = _plan(kblk, wup)
    em = np.asarray(emissions, dtype=F32_NP).copy()
    em[:, 0, :] += np.asarray(start_transitions, dtype=F32_NP)
    em[:, -1, :] += np.asarray(end_transitions, dtype=F32_NP)
    em_b = em.astype(BF16_NP)
    b_all, s_len = em.shape[0], em.shape[1]
    n_chunks = s_len // 128
    L = R - 1 - wup
    # positions per (k, round j)
    P = np.empty((kblk, R), dtype=np.int64)
    P[0] = np.arange(R)
    for k in range(1, kblk):
        a_k = R + (k - 1) * L
        P[k] = a_k - wup - 1 + np.arange(R)
    fx = em_b[:, P, :]                    # [B, K, R, T]
    bx = em_b[:, s_len - 1 - P, :]        # [B, K, R, T]
    X = np.concatenate([fx, bx], axis=3)  # [B, K, R, 2T]
    Rpad = n_xch * CH
    if Rpad > R:
        pad = np.zeros((b_all, kblk, Rpad - R, 2 * T), dtype=BF16_NP)
        X = np.concatenate([X, pad], axis=2)
    # -> [c, t(128), j2, k, b]
    X = X.reshape(b_all, kblk, n_xch, CH, 2 * T)
    X = X.transpose(2, 4, 3, 1, 0)        # [c, 128, CH, K, B]
    # em_m[g, s, b, t] + onehots
    em_m = em_b.reshape(b_all, n_chunks, 128, T).transpose(1, 2, 0, 3)
    tags_i = np.asarray(tags).astype(np.int64).reshape(b_all, s_len)
    onehot = np.zeros((b_all, s_len, T), dtype=BF16_NP)
    np.put_along_axis(onehot, tags_i[:, :, None], BF16_NP(1.0), axis=2)
    oh_m = onehot.reshape(b_all, n_chunks, 128, T).transpose(1, 2, 0, 3)
    trans = np.asarray(transitions, dtype=F32_NP).reshape(T, T)
    blockw = np.zeros((128, 128), dtype=BF16_NP)
    blockw[0:T, 0:T] = np.exp(trans).astype(BF16_NP)
    blockw[T:128, T:128] = np.exp(trans.T).astype(BF16_NP)
    bones, bsel = _consts()
    bsh = b_all // ncores
    m0 = np.zeros((2, kblk, bsh), dtype=F32_NP)
    m0[:, 0, :] = 1.0
    m0 = m0.reshape(2, kblk * bsh)
    m1 = (1.0 - m0).astype(F32_NP)
    in_maps = []
    for cidx in range(ncores):
        sl = slice(cidx * bsh, (cidx + 1) * bsh)
        Xc = X[:, :, :, :, sl].reshape(n_xch, 128, CH, kblk * bsh)
        in_maps.append({
            "emx": np.ascontiguousarray(Xc),
            "emm": np.ascontiguousarray(em_m[:, :, sl, :]),
            "oh": np.ascontiguousarray(oh_m[:, :, sl, :]),
            "blockw": blockw,
            "bones": bones,
            "bsel": bsel,
            "mask0": m0,
            "mask1": m1,
        })
    return in_maps


def _host_trans_part(transitions, tags):
    tags_i = np.asarray(tags).astype(np.int64)
    pair_idx = tags_i[:, :-1] * T + tags_i[:, 1:]
    hist = np.bincount(pair_idx.ravel(), minlength=T * T).reshape(T, T)
    trans = np.asarray(transitions, dtype=np.float64).reshape(T, T)
    return float((hist * trans).sum())


def kernel(emissions, start_transitions, end_transitions, transitions,
           tags, mask):
    """Full-input entry point; shards over 8 NeuronCores internally."""
    from concourse.bass_utils import run_bass_kernel_spmd

    emissions = np.asarray(emissions)
    assert emissions.shape == (B, S, T)
    assert (np.asarray(mask) != 0).all(), "kernel assumes all-ones mask"

    in_maps = make_in_maps(emissions, start_transitions, end_transitions,
                           transitions, tags)
    nc = _get_nc()
    res = run_bass_kernel_spmd(nc, in_maps, core_ids=list(range(NCORES)))

    kblk, bsh = KBLK, BSH
    num_total = _host_trans_part(transitions, tags)
    den_total = 0.0
    for cidx in range(NCORES):
        r = res.results[cidx]
        na = r["numacc"].astype(np.float64)
        num_total += float(np.trace(na) if na.shape == (T, T) else na.sum())
        zall = r["zall"].astype(np.float64).reshape(2, kblk, bsh)
        pacc = r["pacco"].astype(np.float64).reshape(2, kblk, bsh)
        zj = r["zj"].astype(np.float64).reshape(bsh)
        den = np.zeros(bsh, dtype=np.float64)
        for row in range(2):
            for k in range(kblk - 1):
                den += np.log(zall[row, k]) - np.log(pacc[row, k])
            den += -np.log(pacc[row, kblk - 1])
        den += np.log(zj) + float(S) * C0
        den_total += float(den.sum())
    loss = -(num_total - den_total) / float(B)
    return np.float32(loss)


# revision 52
# speedup vs baseline: 61.0541x; 1.4866x over previous
"""CRF negative-log-likelihood loss kernel for Trainium2 (8 NeuronCores).

Problem: B=256, S=2048, T=64 CRF loss (torchcrf-style), mask all-ones.

Strategy (v3: blocked chains)
-----------------------------
Data-parallel over batch: each of the 8 cores gets 32 batch rows.

Denominator (log-partition): forward/backward meet-in-the-middle in the
exp domain, PLUS a sequence-blocking trick that exploits the Hilbert-
metric contraction of positive matrix products: each half (1024 steps)
is split into K blocks that run as INDEPENDENT chains.  Block 0 starts
from the true initial state; blocks k>=1 start w rounds early from an
arbitrary positive state (the emission vector at that position) and
"burn in" -- after w warmup steps the state direction has converged to
the true one within ~0.46^w (validated 3e-6 end-to-end), so the
measured per-step mass growth over the block's window is exact for all
practical purposes.  The measured window's growth is recovered as
  ln(final mass) - ln(mass at warmup boundary) - sum ln(renorms),
with the boundary mass folded into the same per-chain accumulator
`pacc` that logs the bf16 renormalization factors (product form).

All 2K blocks (K fwd + K bwd) advance together in ONE round:
a 128x128 block-diagonal matmul (W / W^T) against a [128, K*32] state,
then one DVE multiply with the per-round emission-exp slice.  Serial
rounds drop from 1023 to R = (1024 + (K-1)(w+1)) / K  (~142 at K=8).

Numerator: onehots built on the HOST and DMA'd; per 128-step chunk one
Pool-engine multiply + reduce against em (no PE/DVE involvement).
Transition part sum trans[tag_s, tag_s+1] depends only on tags ->
host bincount in f64.  start/end transitions are folded into em rows
s=0 / s=S-1 on the host.

All log/assembly math runs on the host in f64 from raw f32 dumps
(zall, pacc, zj, numacc) -- the device never calls Ln.
"""

import contextlib
from collections import deque

import numpy as np
import ml_dtypes

F32_NP = np.float32
BF16_NP = ml_dtypes.bfloat16

B, S, T = 256, 2048, 64
NCORES = 8
BSH = B // NCORES  # 32
C0 = 4.8204  # ~ ln(64 * e^0.5 * sinh(1)) : expected per-step log growth
RN = 128     # renorm every RN rounds (R=79 -> no renorm needed at all)
KBLK = 16    # blocks per direction
WUP = 15     # warmup rounds for blocks >= 1

_NC_CACHE = {}


def _plan(kblk=KBLK, wup=WUP):
    half = S // 2
    R = (half + (kblk - 1) * (wup + 1)) // kblk
    assert R * kblk - (kblk - 1) * (wup + 1) == half, (kblk, wup)
    cols = 2 * T * kblk * BSH // 128  # = kblk*BSH (128-part state)
    ch = max(8, min(32, 8192 // cols))  # x-chunk rounds
    n_xch = -(-R // ch)
    return R, cols, ch, n_xch


def build(nrep=1, fake_x=False, no_num=False, no_rounds=False,
          rn=RN, kblk=KBLK, wup=WUP, pround_bufs=4, spool_bufs=6,
          warm=0, no_ldw_wait=True, num_dve=False, num_pe=False,
          num_gather=True, dmasplit=4, bsh=BSH):
    """Build + compile the per-core Bass module."""
    import concourse.bacc as bacc
    import concourse.mybir as mybir
    import concourse.tile as tile

    F32 = mybir.dt.float32
    BF16 = mybir.dt.bfloat16
    AF = mybir.ActivationFunctionType
    ALU = mybir.AluOpType

    R, cols, CH, n_xch = _plan(kblk, wup)
    n_chunks = S // 128  # numerator chunks

    nc = bacc.Bacc("TRN2", target_bir_lowering=False, debug=False,
                   num_devices=NCORES)
    if no_ldw_wait:
        nc.move_matmul_waits_to_ldweights = lambda: None

    em_x_d = nc.dram_tensor("emx", [n_xch, 128, CH, cols], BF16,
                            kind="ExternalInput")
    if num_gather:
        # host-gathered em[b, s, tag[b,s]] -> [128, n_chunks*bsh] bf16
        gem_d = nc.dram_tensor("gem", [128, n_chunks * bsh], BF16,
                               kind="ExternalInput")
    else:
        em_m_d = nc.dram_tensor("emm", [n_chunks, 128, bsh, T], BF16,
                                kind="ExternalInput")
        oh_d = nc.dram_tensor("oh", [n_chunks, 128, bsh, T], BF16,
                              kind="ExternalInput")
    blockw_d = nc.dram_tensor("blockw", [128, 128], BF16,
                              kind="ExternalInput")
    bones_d = nc.dram_tensor("bones", [128, 2], BF16, kind="ExternalInput")
    bsel_d = nc.dram_tensor("bsel", [2, 128], BF16, kind="ExternalInput")
    mask0_d = nc.dram_tensor("mask0", [2, cols], F32, kind="ExternalInput")
    mask1_d = nc.dram_tensor("mask1", [2, cols], F32, kind="ExternalInput")
    zall_d = nc.dram_tensor("zall", [2, cols], F32, kind="ExternalOutput")
    pacc_d = nc.dram_tensor("pacco", [2, cols], F32, kind="ExternalOutput")
    zj_d = nc.dram_tensor("zj", [1, bsh], F32, kind="ExternalOutput")
    NQ = 8  # numerator sub-quanta per chunk (num_dve mode)
    if num_gather:
        num_d = nc.dram_tensor("numacc", [128, 1], F32,
                               kind="ExternalOutput")
    elif num_pe:
        num_d = nc.dram_tensor("numacc", [T, T], F32,
                               kind="ExternalOutput")
    else:
        num_cols = n_chunks * NQ if num_dve else n_chunks
        num_d = nc.dram_tensor("numacc", [128, num_cols], F32,
                               kind="ExternalOutput")

    with tile.TileContext(nc) as tc, nc.allow_low_precision(
            reason="bf16 state/weights validated against f64 reference"):
        with (
            tc.tile_pool(name="consts", bufs=1) as consts,
            tc.tile_pool(name="xchunk", bufs=2) as xpool,
            tc.tile_pool(name="xraw", bufs=2) as xrawpool,
            tc.tile_pool(name="emt", bufs=4) as empool,
            tc.tile_pool(name="oht", bufs=4) as ohpool,
            tc.tile_pool(name="scr", bufs=2) as scrpool,
            tc.tile_pool(name="state", bufs=spool_bufs) as spool,
            tc.tile_pool(name="rm", bufs=3) as rmpool,
            tc.tile_pool(name="small", bufs=4) as smallpool,
            tc.tile_pool(name="pround", bufs=pround_bufs,
                         space="PSUM") as pround,
            tc.tile_pool(name="pmisc", bufs=1, space="PSUM") as pmisc,
            tc.tile_pool(name="pnum", bufs=1, space="PSUM") as pnum,
        ):
            rep_ctx = (tc.For_i(0, nrep, 1) if nrep > 1
                       else contextlib.nullcontext())
            with rep_ctx:
                # ---------------- constants / setup ----------------
                blockw = consts.tile([128, 128], BF16, tag="blockw")
                nc.sync.dma_start(blockw[:], blockw_d.ap())
                blockones = consts.tile([128, 2], BF16, tag="blockones")
                nc.sync.dma_start(blockones[:], bones_d.ap())
                blocksel = consts.tile([2, 128], BF16, tag="blocksel")
                nc.sync.dma_start(blocksel[:], bsel_d.ap())
                mask0 = consts.tile([2, cols], F32, tag="mask0")
                nc.sync.dma_start(mask0[:], mask0_d.ap())
                mask1 = consts.tile([2, cols], F32, tag="mask1")
                nc.sync.dma_start(mask1[:], mask1_d.ap())
                ones64 = consts.tile([T, 1], F32, tag="ones64")
                nc.vector.memset(ones64[:], 1.0)
                negc0 = consts.tile([128, 1], F32, tag="negc0")
                nc.vector.memset(negc0[:], -C0)

                pacc = consts.tile([2, cols], F32, tag="pacc")
                nc.vector.memset(pacc[:], 1.0)
                if num_gather:
                    gem_sb = consts.tile([128, n_chunks * bsh], BF16,
                                         tag="gem")
                    if not no_num:
                        nc.sync.dma_start(gem_sb[:], gem_d.ap())
                elif num_pe:
                    m_ps = pnum.tile([T, T], F32, tag="mps")
                    num_state = {"first": True, "last": None}
                else:
                    numacc = consts.tile([128, num_cols], F32,
                                         tag="numacc")
                    nc.vector.memset(numacc[:], 0.0)

                emg = {}      # numerator em chunk g
                ohg = {}      # numerator onehot chunk g
                exraw = {}    # x chunk c raw (pre-exp)

                def dma_x(c):
                    xr = xrawpool.tile([128, CH, cols], BF16, tag="xr")
                    step = -(-CH // dmasplit)
                    for lo in range(0, CH, step):
                        hi = min(CH, lo + step)
                        nc.sync.dma_start(xr[:, lo:hi, :],
                                          em_x_d.ap()[c, :, lo:hi, :])
                    exraw[c] = xr

                def dma_num(g):
                    if num_gather:
                        return
                    eg = empool.tile([128, bsh, T], BF16, tag="em")
                    nc.sync.dma_start(eg[:], em_m_d.ap()[g])
                    emg[g] = eg
                    og = ohpool.tile([128, bsh, T], BF16, tag="oh")
                    nc.sync.dma_start(og[:], oh_d.ap()[g])
                    ohg[g] = og

                bgn = deque()  # numerator sub-quanta (num_dve mode)
                bgm = deque()  # numerator matmuls (num_pe mode)

                def num_chunk(g):
                    """Gather-reduce (em*onehot) for chunk g, off-chain."""
                    if num_pe:
                        # one accumulating PE matmul per batch row:
                        # m_ps += onehot_{g,b}^T @ em_{g,b}
                        for b in range(bsh):
                            def q(g=g, b=b, last=(b == bsh - 1)):
                                mm = nc.tensor.matmul(
                                    m_ps[:], ohg[g][:, b, :],
                                    emg[g][:, b, :],
                                    start=num_state["first"], stop=False,
                                    skip_group_check=True)
                                num_state["first"] = False
                                num_state["last"] = mm
                                if last:
                                    del emg[g], ohg[g]
                            bgm.append(q)
                        return
                    use_dve = (num_dve is True) or \
                        (num_dve == "mixed" and g % 2 == 1)
                    if use_dve:
                        # 8 DVE sub-quanta, popped one per round (small
                        # enough to hide in the chain's DVE idle slack)
                        bq = bsh // NQ
                        for qi in range(NQ):
                            sl = slice(qi * bq, (qi + 1) * bq)
                            def q(g=g, qi=qi, sl=sl,
                                  last=(qi == NQ - 1)):
                                scr = scrpool.tile([128, bq, T], BF16,
                                                   tag="scr")
                                nc.vector.scalar_tensor_tensor(
                                    scr[:], emg[g][:, sl, :], 1.0,
                                    ohg[g][:, sl, :],
                                    op0=ALU.bypass, op1=ALU.mult,
                                    accum_out=numacc[:, g * NQ + qi:
                                                     g * NQ + qi + 1])
                                if last:
                                    del emg[g], ohg[g]
                            bgn.append(q)
                    else:
                        gc = g * NQ if num_dve else g
                        scr = scrpool.tile([128, bsh, T], BF16, tag="scr")
                        nc.gpsimd.tensor_tensor(
                            scr[:], emg[g][:], ohg[g][:], op=ALU.mult)
                        nc.gpsimd.tensor_reduce(
                            numacc[0:1, gc:gc + 1], scr[:],
                            axis=mybir.AxisListType.XYZWC, op=ALU.add)
                        del emg[g], ohg[g]

                def x_quanta(c, nrounds):
                    """ACT-exp quanta producing X chunk c from em_x."""
                    xc = xpool.tile([128, CH, cols], BF16, tag="xc")
                    if fake_x:
                        def q():
                            nc.gpsimd.memset(xc[:], 0.0133)
                        return xc, [q]
                    qs = []
                    qn = 4
                    step = -(-nrounds // qn)
                    for hj in range(qn):
                        lo = hj * step
                        hi = min(nrounds, lo + step)
                        if lo >= hi:
                            continue
                        def q(lo=lo, hi=hi, last=(hi >= nrounds)):
                            nc.scalar.activation(
                                xc[:, lo:hi, :], exraw[c][:, lo:hi, :],
                                AF.Exp, bias=negc0[:])
                            if last:
                                del exraw[c]
                        qs.append(q)
                    return xc, qs

                # ---------------- priming ----------------
                bg = deque()
                xchunks = {}
                if not fake_x:
                    dma_x(0)
                    if n_xch > 1:
                        dma_x(1)
                    if not no_num:
                        for g in range(min(3, n_chunks)):
                            dma_num(g)
                for c in (0, 1):
                    if c <= n_xch - 1:
                        nr = min(CH, R - c * CH)
                        xc, qs = x_quanta(c, nr)
                        [q() for q in qs]
                        xchunks[c] = xc

                state = spool.tile([128, cols], BF16, tag="st")
                nc.vector.tensor_copy(state[:], xchunks[0][:, 0, :])

                if warm:
                    wsrc = consts.tile([128, bsh], BF16, tag="wsrc")
                    nc.vector.memset(wsrc[:], 0.5)

                # numerator schedule: chunk g at round 2 + step*g
                r_end = 0 if no_rounds else R - 1
                num_step = max(1, (r_end - 4) // max(1, n_chunks))

                def num_slot(r):
                    if no_num or fake_x or num_gather or r < 2 \
                            or (r - 2) % num_step:
                        return None
                    g = (r - 2) // num_step
                    return g if g < n_chunks else None

                for r in range(1, r_end + 1):
                    c, j = divmod(r, CH)
                    if (j == 1 and not fake_x and c + 1 <= n_xch - 1
                            and (c + 1) not in exraw
                            and (c + 1) not in xchunks):
                        dma_x(c + 1)
                    if j == 2 and c + 1 <= n_xch - 1 and \
                            (c + 1) not in xchunks:
                        nr = min(CH, R - (c + 1) * CH)
                        xc, qs = x_quanta(c + 1, nr)
                        xchunks[c + 1] = xc
                        bg.extend(qs)
                        xchunks.pop(c - 1, None)
                    g = num_slot(r)
                    if g is not None:
                        num_chunk(g)
                        if g + 3 < n_chunks:
                            dma_num(g + 3)
                    if bg:
                        bg.popleft()()
                    if bgn:
                        bgn.popleft()()
                    for _ in range(7):
                        if not bgm:
                            break
                        bgm.popleft()()
                    for _ in range(warm):
                        wps = pround.tile([128, cols], F32, tag="p")
                        nc.tensor.matmul(wps[:, 0:bsh], blockw[:], wsrc[:],
                                         start=True, stop=True,
                                         skip_group_check=True)
                    p = pround.tile([128, cols], F32, tag="p")
                    nc.tensor.matmul(p[:], blockw[:], state[:],
                                     start=True, stop=True)
                    nstate = spool.tile([128, cols], BF16, tag="st")
                    nc.vector.tensor_mul(nstate[:], p[:],
                                         xchunks[c][:, j, :])
                    state = nstate

                    if r == wup:
                        # warmup boundary: pacc := pacc*mask0 + mass*mask1
                        mass = pmisc.tile([2, cols], F32, tag="m2")
                        nc.tensor.matmul(mass[:], blockones[:], state[:],
                                         start=True, stop=True)
                        nc.vector.tensor_mul(pacc[:], pacc[:], mask0[:])
                        bmt = smallpool.tile([2, cols], F32, tag="bmt")
                        nc.vector.scalar_tensor_tensor(
                            bmt[:], mass[:], 1.0, mask1[:],
                            op0=ALU.bypass, op1=ALU.mult)
                        nc.vector.tensor_add(pacc[:], pacc[:], bmt[:])

                    if r % rn == 0 and r < r_end:
                        mass = pmisc.tile([2, cols], F32, tag="m2")
                        nc.tensor.matmul(mass[:], blockones[:], state[:],
                                         start=True, stop=True)
                        rmass = rmpool.tile([2, cols], BF16, tag="rm")
                        nc.vector.reciprocal(rmass[:], mass[:])
                        nc.vector.tensor_mul(pacc[:], pacc[:], rmass[:])
                        rbc = pmisc.tile([128, cols], F32, tag="rbc")
                        nc.tensor.matmul(rbc[:], blocksel[:], rmass[:],
                                         start=True, stop=True)
                        nstate = spool.tile([128, cols], BF16, tag="st")
                        nc.vector.tensor_mul(nstate[:], state[:], rbc[:])
                        state = nstate

                while bg:
                    bg.popleft()()
                while bgn:
                    bgn.popleft()()
                while bgm:
                    bgm.popleft()()
                if no_rounds and not fake_x and not no_num \
                        and not num_gather:
                    for g in range(3, n_chunks):
                        dma_num(g)
                    for g in range(n_chunks):
                        num_chunk(g)
                    while bgn:
                        bgn.popleft()()
                    while bgm:
                        bgm.popleft()()

                # ---------------- final reads ----------------
                zps = pmisc.tile([2, cols], F32, tag="m2")
                nc.tensor.matmul(zps[:], blockones[:], state[:],
                                 start=True, stop=True)
                zall_sb = smallpool.tile([2, cols], F32, tag="zall")
                nc.vector.tensor_copy(zall_sb[:], zps[:])
                nc.sync.dma_start(zall_d.ap(), zall_sb[:])
                nc.sync.dma_start(pacc_d.ap(), pacc[:])

                lastc = slice((kblk - 1) * bsh, kblk * bsh)
                pf = pround.tile([128, cols], F32, tag="p")
                nc.tensor.matmul(pf[0:T, 0:bsh], blockw[T:128, T:128],
                                 state[T:128, lastc], start=True, stop=True)
                y = smallpool.tile([T, bsh], F32, tag="y")
                nc.vector.tensor_mul(y[:], state[0:T, lastc], pf[0:T, 0:bsh])
                zjp = pmisc.tile([2, cols], F32, tag="m2")
                nc.tensor.matmul(zjp[0:1, 0:bsh], ones64[:], y[:],
                                 start=True, stop=True)
                zj_sb = smallpool.tile([1, bsh], F32, tag="zj")
                nc.vector.tensor_copy(zj_sb[:], zjp[0:1, 0:bsh])
                nc.sync.dma_start(zj_d.ap(), zj_sb[:])
                if num_gather:
                    gacc = smallpool.tile([128, 1], F32, tag="gacc")
                    if no_num:
                        nc.vector.memset(gacc[:], 0.0)
                    else:
                        nc.vector.tensor_reduce(
                            gacc[:], gem_sb[:],
                            axis=mybir.AxisListType.XYZW, op=ALU.add)
                    nc.sync.dma_start(num_d.ap(), gacc[:])
                elif num_pe:
                    if num_state["last"] is not None:
                        num_state["last"].ins.stop_tensor_calc = True
                        mps_sb = smallpool.tile([T, T], F32, tag="mpssb")
                        nc.vector.tensor_copy(mps_sb[:], m_ps[:])
                        nc.sync.dma_start(num_d.ap(), mps_sb[:])
                    else:
                        mps_sb = smallpool.tile([T, T], F32, tag="mpssb")
                        nc.vector.memset(mps_sb[:], 0.0)
                        nc.sync.dma_start(num_d.ap(), mps_sb[:])
                else:
                    nc.sync.dma_start(num_d.ap(), numacc[:])

    nc.compile()
    return nc


def _get_nc():
    key = "v3"
    if key not in _NC_CACHE:
        _NC_CACHE[key] = build()
    return _NC_CACHE[key]


def _consts():
    bones = np.zeros((128, 2), dtype=F32_NP)
    bones[0:T, 0] = 1.0
    bones[T:128, 1] = 1.0
    bsel = np.zeros((2, 128), dtype=F32_NP)
    bsel[0, 0:T] = 1.0
    bsel[1, T:128] = 1.0
    return bones.astype(BF16_NP), bsel.astype(BF16_NP)


def make_in_maps(emissions, start_transitions, end_transitions, transitions,
                 tags, ncores=NCORES, kblk=KBLK, wup=WUP, num_gather=True):
    """Host prep: fold start/end into em, convert to bf16, build the
    blocked x layout + numerator layouts, shard over cores."""
    R, cols, CH, n_xch = _plan(kblk, wup)
    em = np.asarray(emissions, dtype=F32_NP).copy()
    em[:, 0, :] += np.asarray(start_transitions, dtype=F32_NP)
    em[:, -1, :] += np.asarray(end_transitions, dtype=F32_NP)
    em_b = em.astype(BF16_NP)
    b_all, s_len = em.shape[0], em.shape[1]
    n_chunks = s_len // 128
    L = R - 1 - wup
    # positions per (k, round j)
    P = np.empty((kblk, R), dtype=np.int64)
    P[0] = np.arange(R)
    for k in range(1, kblk):
        a_k = R + (k - 1) * L
        P[k] = a_k - wup - 1 + np.arange(R)
    fx = em_b[:, P, :]                    # [B, K, R, T]
    bx = em_b[:, s_len - 1 - P, :]        # [B, K, R, T]
    X = np.concatenate([fx, bx], axis=3)  # [B, K, R, 2T]
    Rpad = n_xch * CH
    if Rpad > R:
        pad = np.zeros((b_all, kblk, Rpad - R, 2 * T), dtype=BF16_NP)
        X = np.concatenate([X, pad], axis=2)
    # -> [c, t(128), j2, k, b]
    X = X.reshape(b_all, kblk, n_xch, CH, 2 * T)
    X = X.transpose(2, 4, 3, 1, 0)        # [c, 128, CH, K, B]
    tags_i = np.asarray(tags).astype(np.int64).reshape(b_all, s_len)
    if num_gather:
        # host-gathered em[b, s, tag] (bf16 -> device sums in f32)
        gem = np.take_along_axis(em_b, tags_i[:, :, None], axis=2)[:, :, 0]
        gem_m = gem.reshape(b_all, n_chunks, 128).transpose(2, 1, 0)
    else:
        # em_m[g, s, b, t] + onehots
        em_m = em_b.reshape(b_all, n_chunks, 128, T).transpose(1, 2, 0, 3)
        onehot = np.zeros((b_all, s_len, T), dtype=BF16_NP)
        np.put_along_axis(onehot, tags_i[:, :, None], BF16_NP(1.0), axis=2)
        oh_m = onehot.reshape(b_all, n_chunks, 128, T).transpose(1, 2, 0, 3)
    trans = np.asarray(transitions, dtype=F32_NP).reshape(T, T)
    blockw = np.zeros((128, 128), dtype=BF16_NP)
    blockw[0:T, 0:T] = np.exp(trans).astype(BF16_NP)
    blockw[T:128, T:128] = np.exp(trans.T).astype(BF16_NP)
    bones, bsel = _consts()
    bsh = b_all // ncores
    m0 = np.zeros((2, kblk, bsh), dtype=F32_NP)
    m0[:, 0, :] = 1.0
    m0 = m0.reshape(2, kblk * bsh)
    m1 = (1.0 - m0).astype(F32_NP)
    in_maps = []
    for cidx in range(ncores):
        sl = slice(cidx * bsh, (cidx + 1) * bsh)
        Xc = X[:, :, :, :, sl].reshape(n_xch, 128, CH, kblk * bsh)
        im = {
            "emx": np.ascontiguousarray(Xc),
            "blockw": blockw,
            "bones": bones,
            "bsel": bsel,
            "mask0": m0,
            "mask1": m1,
        }
        if num_gather:
            im["gem"] = np.ascontiguousarray(
                gem_m[:, :, sl].reshape(128, n_chunks * bsh))
        else:
            im["emm"] = np.ascontiguousarray(em_m[:, :, sl, :])
            im["oh"] = np.ascontiguousarray(oh_m[:, :, sl, :])
        in_maps.append(im)
    return in_maps


def _host_trans_part(transitions, tags):
    tags_i = np.asarray(tags).astype(np.int64)
    pair_idx = tags_i[:, :-1] * T + tags_i[:, 1:]
    hist = np.bincount(pair_idx.ravel(), minlength=T * T).reshape(T, T)
    trans = np.asarray(transitions, dtype=np.float64).reshape(T, T)
    return float((hist * trans).sum())


def kernel(emissions, start_transitions, end_transitions, transitions,
           tags, mask):
    """Full-input entry point; shards over 8 NeuronCores internally."""
    from concourse.bass_utils import run_bass_kernel_spmd

    emissions = np.asarray(emissions)
    assert emissions.shape == (B, S, T)
    assert (np.asarray(mask) != 0).all(), "kernel assumes all-ones mask"

    in_maps = make_in_maps(emissions, start_transitions, end_transitions,
                           transitions, tags)
    nc = _get_nc()
    res = run_bass_kernel_spmd(nc, in_maps, core_ids=list(range(NCORES)))

    kblk, bsh = KBLK, BSH
    num_total = _host_trans_part(transitions, tags)
    den_total = 0.0
    for cidx in range(NCORES):
        r = res.results[cidx]
        na = r["numacc"].astype(np.float64)
        num_total += float(np.trace(na) if na.shape == (T, T) else na.sum())
        zall = r["zall"].astype(np.float64).reshape(2, kblk, bsh)
        pacc = r["pacco"].astype(np.float64).reshape(2, kblk, bsh)
        zj = r["zj"].astype(np.float64).reshape(bsh)
        den = np.zeros(bsh, dtype=np.float64)
        for row in range(2):
            for k in range(kblk - 1):
                den += np.log(zall[row, k]) - np.log(pacc[row, k])
            den += -np.log(pacc[row, kblk - 1])
        den += np.log(zj) + float(S) * C0
        den_total += float(den.sum())
    loss = -(num_total - den_total) / float(B)
    return np.float32(loss)
